# revision 1
# baseline (speedup 1.0000x reference)
"""Trainium2 Bass kernel for nn_CombinedModel (3-relation GNN with Bernstein
polynomial message passing).

Self-contained: takes full inputs, shards nodes across 8 NeuronCores,
runs a Bass/Tile SPMD program (MLP -> 2 hops of normalized-Laplacian
aggregation -> polynomial projection), gathers the full output.
"""
import math
import os
from contextlib import ExitStack

import numpy as np

import concourse.bacc as bacc
import concourse.tile as tile
from concourse import mybir
from concourse.bass_utils import run_bass_kernel_spmd
from concourse.masks import make_identity

F16, F32 = mybir.dt.float16, mybir.dt.float32
I16, I32 = mybir.dt.int16, mybir.dt.int32

NCORES = 8
P = 128
H = 128
IN_FEATS = 256
R = 3
D_ORDER = 2
KORD = D_ORDER + 1
WIN_PER_SEG = 4     # windows per gather segment == windows per PSUM bank
MLP_CHUNK = 512


def _bernstein_thetas(d):
    thetas = []
    for i in range(d + 1):
        a = np.zeros(i + 1)
        a[i] = 0.5 ** i
        b = np.array([math.comb(d - i, j) * (-0.5) ** j for j in range(d - i + 1)])
        scale = math.factorial(d + 1) / (math.factorial(i) * math.factorial(d - i))
        thetas.append((np.convolve(a, b) * scale).astype(np.float32))
    return np.stack(thetas)  # [d+1, d+1]


THETAS = _bernstein_thetas(D_ORDER)


# ----------------------------------------------------------------------------
# Host-side preprocessing
# ----------------------------------------------------------------------------

def _make_plan(n):
    nloc = ((n + NCORES * P - 1) // (NCORES * P)) * P
    npad = nloc * NCORES
    assert npad % 2 == 0 and npad // 2 % P == 0
    half = npad // 2
    assert half <= 32768 and npad - half <= 32768
    nwin = nloc // P
    segs = [list(range(s, min(s + WIN_PER_SEG, nwin)))
            for s in range(0, nwin, WIN_PER_SEG)]
    return dict(N=n, NLOC=nloc, NPAD=npad, HALF=half, NWIN=nwin, segs=segs)


def _build_perm(degs_total, npad):
    """Balanced node -> slot permutation. Snake-deal nodes (sorted by total
    degree desc) across all (core, window) bins so per-window edge counts are
    near-equal across cores."""
    n = len(degs_total)
    nloc = npad // NCORES
    nwin_total = npad // P
    tot = np.zeros(npad, np.int64)
    tot[:n] = degs_total
    order = np.argsort(-tot, kind="stable")
    slot_of = np.empty(npad, np.int64)
    counts = np.zeros(nwin_total, np.int64)
    i = np.arange(npad)
    rnd, pos = np.divmod(i, nwin_total)
    w = np.where(rnd % 2 == 0, pos, nwin_total - 1 - pos)
    core = w % NCORES
    j = w // NCORES
    # counts per window in dealing order
    for idx in range(npad):
        g = order[idx]
        ww = w[idx]
        slot_of[g] = core[idx] * nloc + j[idx] * P + counts[ww]
        counts[ww] += 1
    return slot_of


def _pack_idx(idx_flat):
    """[L] int16 -> wrapped [128, L//16] layout (16-partition wrap, replicated)."""
    L = len(idx_flat)
    assert L % 16 == 0
    base = idx_flat.reshape(L // 16, 16).T  # [16, L/16]
    return np.ascontiguousarray(np.tile(base, (8, 1))).astype(np.int16)


def _build_streams(plan, s_slot, d_slot, wgt):
    """Build per-core gather/one-hot streams for one relation.

    Returns (T [NWIN,2] uniform tile counts, per-core dict of streams).
    Stream tile order: for seg in segs: for part in (0,1): for j in seg:
    T[j,part] tiles.
    """
    NLOC, NWIN, HALF = plan["NLOC"], plan["NWIN"], plan["HALF"]
    core = d_slot // NLOC
    j = (d_slot % NLOC) // P
    off = (d_slot % P).astype(np.float32)
    part = (s_slot >= HALF).astype(np.int64)
    key = (core * NWIN + j) * 2 + part
    ngroups = NCORES * NWIN * 2
    cnt = np.bincount(key, minlength=ngroups).reshape(NCORES, NWIN, 2)
    T = np.ceil(cnt / P).astype(np.int64).max(axis=0)  # [NWIN, 2]
    order = np.argsort(key, kind="stable")
    # group start offsets in `order`
    gstart = np.zeros(ngroups + 1, np.int64)
    np.cumsum(np.bincount(key, minlength=ngroups), out=gstart[1:])

    ntiles = int(T.sum())
    # tile offsets per (j, part) in stream order
    tile_off = {}
    tpos = 0
    for seg in plan["segs"]:
        for pt in (0, 1):
            for jj in seg:
                tile_off[(jj, pt)] = tpos
                tpos += int(T[jj, pt])
    assert tpos == ntiles

    idx_vals = s_slot - part * HALF  # int64, fits int16
    # sort edges within each group by source slot: gather descriptors then
    # read the table in ascending address order (HBM row locality)
    order = order[np.lexsort((s_slot[order], key[order]))]
    per_core = []
    for c in range(NCORES):
        idx_c = np.zeros(ntiles * P, np.int64)
        dq_c = np.full(ntiles * P, -1.0, np.float32)
        wq_c = np.zeros(ntiles * P, np.float32)
        for jj in range(NWIN):
            for pt in (0, 1):
                g = (c * NWIN + jj) * 2 + pt
                e = order[gstart[g]:gstart[g + 1]]
                L = len(e)
                if L == 0:
                    continue
                s0 = tile_off[(jj, pt)] * P
                idx_c[s0:s0 + L] = idx_vals[e]
                dq_c[s0:s0 + L] = off[e]
                wq_c[s0:s0 + L] = wgt[e]
        per_core.append(dict(
            idx=_pack_idx(idx_c.astype(np.int16)),
            dq=np.ascontiguousarray(dq_c.reshape(ntiles, P).T),
            wq=np.ascontiguousarray(wq_c.reshape(ntiles, P).T),
        ))
    return T, ntiles, per_core


def preprocess(inputs):
    x = np.asarray(inputs["x"], np.float32)
    n = x.shape[0]
    plan = _make_plan(n)
    NLOC, NPAD = plan["NLOC"], plan["NPAD"]

    srcs, dsts, degs = [], [], []
    for r in range(R):
        s = np.asarray(inputs[f"src{r}"]).astype(np.int64)
        d = np.asarray(inputs[f"dst{r}"]).astype(np.int64)
        srcs.append(s)
        dsts.append(d)
        degs.append(np.bincount(d, minlength=n).astype(np.float64))
    perm = _build_perm(sum(degs)[:n].astype(np.int64), NPAD)  # global -> slot

    meta = dict(N=n, NLOC=NLOC, NPAD=NPAD, HALF=plan["HALF"], NWIN=plan["NWIN"],
                segs=tuple(tuple(s) for s in plan["segs"]))
    Ts, ntiles_l, streams = [], [], []
    for r in range(R):
        dinv = 1.0 / np.sqrt(np.maximum(degs[r], 1.0))
        wgt = (dinv[srcs[r]] * dinv[dsts[r]]).astype(np.float32)
        T, ntiles, per_core = _build_streams(
            plan, perm[srcs[r]], perm[dsts[r]], wgt)
        Ts.append(tuple(tuple(int(v) for v in row) for row in T))
        ntiles_l.append(ntiles)
        streams.append(per_core)
    meta["T"] = tuple(Ts)
    meta["ntiles"] = tuple(ntiles_l)

    # x slices, permuted + transposed per core
    x_slots = np.zeros((NPAD, IN_FEATS), np.float32)
    x_slots[perm[:n]] = x
    in_maps = []
    weight_names = []
    for r in range(R):
        weight_names += [f"W1_{r}", f"b1_{r}", f"W2_{r}", f"b2_{r}"]
    weight_names += ["W3", "b3"]
    for c in range(NCORES):
        m = {"xT": np.ascontiguousarray(
            x_slots[c * NLOC:(c + 1) * NLOC].T)}
        for name in weight_names:
            m[name] = np.asarray(inputs[name], np.float32)
        for r in range(R):
            m[f"idx{r}"] = streams[r][c]["idx"]
            m[f"dq{r}"] = streams[r][c]["dq"]
            m[f"wq{r}"] = streams[r][c]["wq"]
        in_maps.append(m)
    return meta, in_maps, perm


# ----------------------------------------------------------------------------
# Device program
# ----------------------------------------------------------------------------

def build_program(meta):
    NLOC, NPAD, HALF, NWIN = meta["NLOC"], meta["NPAD"], meta["HALF"], meta["NWIN"]
    segs = [list(s) for s in meta["segs"]]
    Ts = [np.array(t, np.int64) for t in meta["T"]]
    ntiles = meta["ntiles"]

    # max tiles in one gather call (seg x part), for V pool sizing
    maxcall = 1
    for r in range(R):
        for seg in segs:
            for pt in (0, 1):
                maxcall = max(maxcall, int(Ts[r][seg, pt].sum()))
    max_ntiles = max(ntiles)

    nc = bacc.Bacc("TRN2", target_bir_lowering=False, debug=False,
                   num_devices=NCORES)

    xT_d = nc.dram_tensor("xT", [IN_FEATS, NLOC], F32, kind="ExternalInput").ap()
    Wd = {}
    for r in range(R):
        Wd[f"W1_{r}"] = nc.dram_tensor(f"W1_{r}", [IN_FEATS, H], F32, kind="ExternalInput").ap()
        Wd[f"b1_{r}"] = nc.dram_tensor(f"b1_{r}", [H], F32, kind="ExternalInput").ap()
        Wd[f"W2_{r}"] = nc.dram_tensor(f"W2_{r}", [H, H], F32, kind="ExternalInput").ap()
        Wd[f"b2_{r}"] = nc.dram_tensor(f"b2_{r}", [H], F32, kind="ExternalInput").ap()
    W3_d = nc.dram_tensor("W3", [KORD * H, H], F32, kind="ExternalInput").ap()
    b3_d = nc.dram_tensor("b3", [H], F32, kind="ExternalInput").ap()
    idx_d, dq_d, wq_d = [], [], []
    for r in range(R):
        idx_d.append(nc.dram_tensor(f"idx{r}", [P, ntiles[r] * 8], I16, kind="ExternalInput").ap())
        dq_d.append(nc.dram_tensor(f"dq{r}", [P, ntiles[r]], F32, kind="ExternalInput").ap())
        wq_d.append(nc.dram_tensor(f"wq{r}", [P, ntiles[r]], F32, kind="ExternalInput").ap())
    out_d = nc.dram_tensor("out", [P, NLOC], F32, kind="ExternalOutput").ap()

    # internal DRAM: AG inputs + shared tables (2 per relation)
    aghin, htab, agtin, ttab = [], [], [], []
    for r in range(R):
        aghin.append(nc.dram_tensor(f"aghin{r}", [NLOC, H], F16))
        htab.append(nc.dram_tensor(f"htab{r}", [NPAD, H], F16, addr_space="Shared"))
        agtin.append(nc.dram_tensor(f"agtin{r}", [NLOC, H], F16))
        ttab.append(nc.dram_tensor(f"ttab{r}", [NPAD, H], F16, addr_space="Shared"))

    mlp_chunks = []
    c0 = 0
    while c0 < NLOC:
        cw = min(MLP_CHUNK, NLOC - c0)
        mlp_chunks.append((c0, cw))
        c0 += cw

    with tile.TileContext(nc) as tc, ExitStack() as ctx:
        consts = ctx.enter_context(tc.tile_pool(name="consts", bufs=1))
        wtmp_p = ctx.enter_context(tc.tile_pool(name="wtmp", bufs=2))
        ht_p = ctx.enter_context(tc.tile_pool(name="ht", bufs=2))
        t1_p = ctx.enter_context(tc.tile_pool(name="t1", bufs=1))
        t2_p = ctx.enter_context(tc.tile_pool(name="t2", bufs=1))
        nm_p = ctx.enter_context(tc.tile_pool(name="nm", bufs=1))
        idx_p = ctx.enter_context(tc.tile_pool(name="idxp", bufs=1))
        dq_p = ctx.enter_context(tc.tile_pool(name="dqp", bufs=1))
        v_p = ctx.enter_context(tc.tile_pool(name="vp", bufs=3))
        m_p = ctx.enter_context(tc.tile_pool(name="mp", bufs=6))
        h1_p = ctx.enter_context(tc.tile_pool(name="h1p", bufs=3))
        oc_p = ctx.enter_context(tc.tile_pool(name="ocp", bufs=2))
        pp_big = ctx.enter_context(tc.tile_pool(name="ppbig", bufs=3, space="PSUM"))
        pp_hop = ctx.enter_context(tc.tile_pool(name="pphop", bufs=3, space="PSUM"))
        pp_tr = ctx.enter_context(tc.tile_pool(name="pptr", bufs=2, space="PSUM"))

        # ---- constants ----
        iota_i = consts.tile([P, P], I32, tag="iotai")
        nc.gpsimd.iota(iota_i[:], pattern=[[1, P]], base=0, channel_multiplier=0)
        iota_f = consts.tile([P, P], F16, tag="iotaf")
        nc.vector.tensor_copy(iota_f[:], iota_i[:])
        ident = consts.tile([P, P], F16, tag="ident")
        make_identity(nc, ident[:])
        zeroM = consts.tile([P, P], F16, tag="zerom")
        nc.vector.memset(zeroM[:], 0.0)

        cast_p = ctx.enter_context(tc.tile_pool(name="castp", bufs=2))

        def load_cast(dst, src_ap, n):
            c0 = 0
            while c0 < n:
                cw = min(1024, n - c0)
                tmp = cast_p.tile([P, 1024], F32, tag="cast")
                nc.sync.dma_start(out=tmp[:, 0:cw], in_=src_ap[:, c0:c0 + cw])
                nc.any.tensor_copy(dst[:, c0:c0 + cw], tmp[:, 0:cw])
                c0 += cw

        # x^T cast to fp16
        xT0 = consts.tile([P, NLOC], F16, tag="xt0")
        xT1 = consts.tile([P, NLOC], F16, tag="xt1")
        load_cast(xT0, xT_d[0:P, :], NLOC)
        load_cast(xT1, xT_d[P:2 * P, :], NLOC)

        # weights (cast fp16 in DMA); biases fp32
        W1a, W1b, W2sb, b1c, b2c = [], [], [], [], []
        for r in range(R):
            wa = consts.tile([P, H], F16, tag=f"w1a{r}")
            wb = consts.tile([P, H], F16, tag=f"w1b{r}")
            w2 = consts.tile([P, H], F16, tag=f"w2{r}")
            load_cast(wa, Wd[f"W1_{r}"][0:P, :], H)
            load_cast(wb, Wd[f"W1_{r}"][P:2 * P, :], H)
            load_cast(w2, Wd[f"W2_{r}"][:, :], H)
            b1 = consts.tile([P, 1], F32, tag=f"b1{r}")
            b2 = consts.tile([P, 1], F32, tag=f"b2{r}")
            nc.sync.dma_start(out=b1[:], in_=Wd[f"b1_{r}"][:, None])
            nc.sync.dma_start(out=b2[:], in_=Wd[f"b2_{r}"][:, None])
            W1a.append(wa); W1b.append(wb); W2sb.append(w2)
            b1c.append(b1); b2c.append(b2)

        # W3 folded by Bernstein thetas: W3p_k = sum_j THETA[j,k] * W3_j
        w3s = []
        for jj in range(KORD):
            t = wtmp_p.tile([P, H], F32, tag=f"w3s{jj}")
            nc.sync.dma_start(out=t[:], in_=W3_d[jj * H:(jj + 1) * H, :])
            w3s.append(t)
        W3p = []
        for k in range(KORD):
            acc = wtmp_p.tile([P, H], F32, tag=f"w3acc{k}")
            nc.vector.tensor_scalar(out=acc[:], in0=w3s[0][:],
                                    scalar1=float(THETAS[0, k]), scalar2=None,
                                    op0=mybir.AluOpType.mult)
            for jj in range(1, KORD):
                t2t = wtmp_p.tile([P, H], F32, tag="w3mul")
                nc.vector.tensor_scalar(out=t2t[:], in0=w3s[jj][:],
                                        scalar1=float(THETAS[jj, k]), scalar2=None,
                                        op0=mybir.AluOpType.mult)
                nc.vector.tensor_tensor(out=acc[:], in0=acc[:], in1=t2t[:],
                                        op=mybir.AluOpType.add)
            wk = consts.tile([P, H], F16, tag=f"w3p{k}")
            nc.vector.tensor_copy(wk[:], acc[:])
            W3p.append(wk)
        b3x3 = consts.tile([P, 1], F32, tag="b3x3")
        nc.sync.dma_start(out=b3x3[:], in_=b3_d[:, None])
        nc.vector.tensor_scalar(out=b3x3[:], in0=b3x3[:], scalar1=3.0,
                                scalar2=None, op0=mybir.AluOpType.mult)

        out_acc = consts.tile([P, NLOC], F32, tag="outacc")

        def transpose_to_nm(src_fm, nm_tile):
            if os.environ.get("KSKIP_TR"):
                nc.any.tensor_copy(nm_tile[:], src_fm[:])
                return
            for j in range(NWIN):
                tp = pp_tr.tile([P, P], F16, space="PSUM", tag="tr")
                nc.tensor.transpose(out=tp[:], in_=src_fm[:, j * P:(j + 1) * P],
                                    identity=ident[:])
                nc.any.tensor_copy(nm_tile[:, j * P:(j + 1) * P], tp[:])

        def store_and_allgather(nm_tile, ag_in, table):
            nc.sync.dma_start(
                out=ag_in.ap().rearrange("(j p) f -> p j f", p=P),
                in_=nm_tile[:].rearrange("p (j f) -> p j f", f=H))
            nc.gpsimd.collective_compute(
                "AllGather", mybir.AluOpType.bypass,
                ins=[ag_in.ap()], outs=[table.ap()],
                replica_groups=[list(range(NCORES))])

        def hop(r, T, table, prev_fm, next_fm, idx_sb, dq_sb, wq_sb):
            """next_fm = prev_fm - A_hat @ gathered(table)."""
            kmode = os.environ.get("KMODE", "full")
            if kmode == "nohop":
                nc.any.tensor_copy(next_fm[:], prev_fm[:])
                return
            glimit = int(os.environ.get("KGLIMIT", "1000000"))
            lo = table.ap()[0:HALF, :]
            hi = table.ap()[HALF:NPAD, :]
            icol = 0      # idx_sb column offset (8 cols per tile)
            # stream column offset per (window, part), matching host layout:
            # for seg: for part: for j in seg
            stream_off = {}
            tpos = 0
            for seg in segs:
                for pt in (0, 1):
                    for jj in seg:
                        stream_off[(jj, pt)] = tpos
                        tpos += int(T[jj, pt])
            ksp1 = bool(os.environ.get("KSP1"))
            for seg in segs:
                vbufs = {}
                slot0 = {}
                win_vb = {}
                for pt, base in ((0, lo), (1, hi)):
                    if ksp1:
                        # one gather call per (window, part), single_packet
                        # when small enough
                        for jj in seg:
                            tc1 = int(T[jj, pt])
                            if tc1 == 0:
                                continue
                            vbw = v_p.tile([P, maxcall * P], F16, tag="vbuf")
                            nc.gpsimd.dma_gather(
                                out_ap=vbw[:, 0:tc1 * P].rearrange(
                                    "p (t e) -> p t e", e=P),
                                in_ap=base,
                                idxs_ap=idx_sb[:, icol:icol + tc1 * 8],
                                num_idxs=tc1 * P,
                                num_idxs_reg=tc1 * P,
                                elem_size=H,
                                single_packet=(tc1 * P <= 1024),
                            )
                            icol += tc1 * 8
                            win_vb[(jj, pt)] = vbw
                        continue
                    tcount = int(T[seg, pt].sum())
                    if tcount == 0:
                        continue
                    vb = v_p.tile([P, maxcall * P], F16, tag="vbuf")
                    nc._gcount = getattr(nc, "_gcount", 0)
                    use_memset = (kmode == "nogather") or (nc._gcount >= glimit)
                    nc._gcount += 1
                    if use_memset:
                        nc.vector.memset(vb[:, 0:tcount * P], 0.0)
                        icol += tcount * 8
                        vbufs[pt] = vb
                        s_ = 0
                        for jj in seg:
                            slot0[(jj, pt)] = s_
                            s_ += int(T[jj, pt])
                        continue
                    nc.gpsimd.dma_gather(
                        out_ap=vb[:, 0:tcount * P].rearrange(
                            "p (t e) -> p t e", e=P),
                        in_ap=base,
                        idxs_ap=idx_sb[:, icol:icol + tcount * 8],
                        num_idxs=tcount * P,
                        num_idxs_reg=tcount * P,
                        elem_size=H,
                        single_packet=False,
                    )
                    icol += tcount * 8
                    vbufs[pt] = vb
                    # slot base per window within this call
                    s = 0
                    for jj in seg:
                        slot0[(jj, pt)] = s
                        s += int(T[jj, pt])
                j0 = seg[0]
                bw = len(seg)
                sep = bool(os.environ.get("KSEP_PSUM"))
                if not sep:
                    ps = pp_hop.tile([P, WIN_PER_SEG * P], F32, space="PSUM", tag="hop")
                win_ps = {}
                for jj in seg:
                    tot = int(T[jj, 0] + T[jj, 1])
                    if sep:
                        wps = pp_hop.tile([P, P], F32, space="PSUM", tag="hopsep")
                        win_ps[jj] = wps
                        reg = wps[:]
                    else:
                        reg = ps[:, (jj - j0) * P:(jj - j0 + 1) * P]
                    if tot == 0:
                        nc.tensor.matmul(out=reg, lhsT=zeroM[:], rhs=zeroM[:],
                                         start=True, stop=True)
                        continue
                    k = 0
                    for pt in (0, 1):
                        for t in range(int(T[jj, pt])):
                            col = stream_off[(jj, pt)] + t
                            m = m_p.tile([P, P], F16, tag="onehot")
                            nc.any.tensor_scalar(
                                out=m[:], in0=iota_f[:],
                                scalar1=dq_sb[:, col:col + 1],
                                scalar2=wq_sb[:, col:col + 1],
                                op0=mybir.AluOpType.is_equal,
                                op1=mybir.AluOpType.mult)
                            if ksp1:
                                vsrc = win_vb[(jj, pt)]
                                sl = t
                            else:
                                vsrc = vbufs[pt]
                                sl = slot0[(jj, pt)] + t
                            nc.tensor.matmul(
                                out=reg,
                                lhsT=vsrc[:, sl * P:(sl + 1) * P],
                                rhs=m[:],
                                start=(k == 0), stop=(k == tot - 1))
                            k += 1
                # epilogue
                if sep:
                    for jj in seg:
                        nc.any.tensor_tensor(
                            out=next_fm[:, jj * P:(jj + 1) * P],
                            in0=prev_fm[:, jj * P:(jj + 1) * P],
                            in1=win_ps[jj][:],
                            op=mybir.AluOpType.subtract)
                else:
                    nc.any.tensor_tensor(
                        out=next_fm[:, j0 * P:(j0 + bw) * P],
                        in0=prev_fm[:, j0 * P:(j0 + bw) * P],
                        in1=ps[:, 0:bw * P],
                        op=mybir.AluOpType.subtract)

        # ---- relations ----
        for r in range(R):
            T = Ts[r]
            idx_sb = idx_p.tile([P, max_ntiles * 8], I16, tag="idx")
            nc.sync.dma_start(out=idx_sb[:, 0:ntiles[r] * 8], in_=idx_d[r][:])
            dq_sb = dq_p.tile([P, max_ntiles], F32, tag="dq")
            wq_sb = dq_p.tile([P, max_ntiles], F32, tag="wq")
            nc.sync.dma_start(out=dq_sb[:, 0:ntiles[r]], in_=dq_d[r][:])
            nc.sync.dma_start(out=wq_sb[:, 0:ntiles[r]], in_=wq_d[r][:])

            hT = ht_p.tile([P, NLOC], F16, tag="ht")
            for (c0, cw) in mlp_chunks:
                ps1 = pp_big.tile([P, MLP_CHUNK], F32, space="PSUM", tag="big")
                nc.tensor.matmul(out=ps1[:, 0:cw], lhsT=W1a[r][:],
                                 rhs=xT0[:, c0:c0 + cw], start=True, stop=False)
                nc.tensor.matmul(out=ps1[:, 0:cw], lhsT=W1b[r][:],
                                 rhs=xT1[:, c0:c0 + cw], start=False, stop=True)
                h1 = h1_p.tile([P, MLP_CHUNK], F16, tag="h1")
                nc.scalar.activation(h1[:, 0:cw], ps1[:, 0:cw],
                                     mybir.ActivationFunctionType.Lrelu,
                                     bias=b1c[r][:], scale=1.0, alpha=0.01)
                ps2 = pp_big.tile([P, MLP_CHUNK], F32, space="PSUM", tag="big")
                nc.tensor.matmul(out=ps2[:, 0:cw], lhsT=W2sb[r][:],
                                 rhs=h1[:, 0:cw], start=True, stop=True)
                nc.scalar.activation(hT[:, c0:c0 + cw], ps2[:, 0:cw],
                                     mybir.ActivationFunctionType.Lrelu,
                                     bias=b2c[r][:], scale=1.0, alpha=0.01)

            nm = nm_p.tile([P, NLOC], F16, tag="nm")
            transpose_to_nm(hT, nm)
            store_and_allgather(nm, aghin[r], htab[r])

            T1 = t1_p.tile([P, NLOC], F16, tag="t1")
            hop(r, T, htab[r], hT, T1, idx_sb, dq_sb, wq_sb)

            nm2 = nm_p.tile([P, NLOC], F16, tag="nm")
            transpose_to_nm(T1, nm2)
            store_and_allgather(nm2, agtin[r], ttab[r])

            T2 = t2_p.tile([P, NLOC], F16, tag="t2")
            hop(r, T, ttab[r], T1, T2, idx_sb, dq_sb, wq_sb)

            for (c0, cw) in mlp_chunks:
                psf = pp_big.tile([P, MLP_CHUNK], F32, space="PSUM", tag="big")
                nc.tensor.matmul(out=psf[:, 0:cw], lhsT=W3p[0][:],
                                 rhs=hT[:, c0:c0 + cw], start=True, stop=False)
                nc.tensor.matmul(out=psf[:, 0:cw], lhsT=W3p[1][:],
                                 rhs=T1[:, c0:c0 + cw], start=False, stop=False)
                nc.tensor.matmul(out=psf[:, 0:cw], lhsT=W3p[2][:],
                                 rhs=T2[:, c0:c0 + cw], start=False, stop=True)
                if r == 0:
                    nc.any.tensor_copy(out_acc[:, c0:c0 + cw], psf[:, 0:cw])
                else:
                    nc.any.tensor_tensor(out=out_acc[:, c0:c0 + cw],
                                         in0=out_acc[:, c0:c0 + cw],
                                         in1=psf[:, 0:cw],
                                         op=mybir.AluOpType.add)

        # ---- output: leaky(out_acc + 3*b3), feat-major ----
        for (c0, cw) in mlp_chunks:
            oc = oc_p.tile([P, MLP_CHUNK], F32, tag="oc")
            nc.scalar.activation(oc[:, 0:cw], out_acc[:, c0:c0 + cw],
                                 mybir.ActivationFunctionType.Lrelu,
                                 bias=b3x3[:], scale=1.0, alpha=0.01)
            nc.sync.dma_start(out=out_d[:, c0:c0 + cw], in_=oc[:, 0:cw])

    nc.compile()
    return nc


# ----------------------------------------------------------------------------
# Entry point
# ----------------------------------------------------------------------------

_prog_cache = {}


def kernel(**inputs):
    meta, in_maps, perm = preprocess(inputs)
    key = repr((meta["N"], meta["NLOC"], meta["T"], meta["ntiles"]))
    if key not in _prog_cache:
        _prog_cache[key] = build_program(meta)
    nc = _prog_cache[key]
    res = run_bass_kernel_spmd(nc, in_maps, list(range(NCORES)))
    outs = [res.results[c]["out"] for c in range(NCORES)]  # [P, NLOC] each
    out_slots = np.concatenate(outs, axis=1).T             # [NPAD, H]
    n = meta["N"]
    return np.ascontiguousarray(out_slots[perm[:n]]).astype(np.float32)



# revision 6
# speedup vs baseline: 1.0279x; 1.0279x over previous
"""Trainium2 Bass kernel for nn_CombinedModel (3-relation GNN with Bernstein
polynomial message passing).

Self-contained: takes full inputs, shards nodes across 8 NeuronCores,
runs a Bass/Tile SPMD program (MLP -> 2 hops of normalized-Laplacian
aggregation -> polynomial projection), gathers the full output.

The node table used by the hop gathers is AllGathered in K chunks so the
collective pipelines against MLP / gather / matmul compute instead of
serializing in front of each hop.
"""
import math
import os
from contextlib import ExitStack

import numpy as np

import concourse.bacc as bacc
import concourse.tile as tile
from concourse import mybir
from concourse.bass_utils import run_bass_kernel_spmd
from concourse.masks import make_identity

F16, F32 = mybir.dt.float16, mybir.dt.float32
I16, I32 = mybir.dt.int16, mybir.dt.int32

NCORES = 8
P = 128
H = 128
IN_FEATS = 256
R = 3
D_ORDER = 2
KORD = D_ORDER + 1
WIN_PER_SEG = 4     # windows per gather segment == windows per PSUM bank
MLP_CHUNK = 512
NCHUNK = int(os.environ.get("KCHUNKS", "2"))   # table / AllGather chunks


def _bernstein_thetas(d):
    thetas = []
    for i in range(d + 1):
        a = np.zeros(i + 1)
        a[i] = 0.5 ** i
        b = np.array([math.comb(d - i, j) * (-0.5) ** j for j in range(d - i + 1)])
        scale = math.factorial(d + 1) / (math.factorial(i) * math.factorial(d - i))
        thetas.append((np.convolve(a, b) * scale).astype(np.float32))
    return np.stack(thetas)  # [d+1, d+1]


THETAS = _bernstein_thetas(D_ORDER)


# ----------------------------------------------------------------------------
# Host-side preprocessing
# ----------------------------------------------------------------------------

def _make_plan(n):
    nloc = ((n + NCORES * P - 1) // (NCORES * P)) * P
    npad = nloc * NCORES
    nwin = nloc // P
    segs = [list(range(s, min(s + WIN_PER_SEG, nwin)))
            for s in range(0, nwin, WIN_PER_SEG)]
    # chunks: K groups of consecutive segs, window counts as equal as possible
    nseg = len(segs)
    bounds = [round(i * nseg / NCHUNK) for i in range(NCHUNK + 1)]
    seg_groups = [list(range(bounds[i], bounds[i + 1])) for i in range(NCHUNK)]
    chunks = []  # (win_start, win_count) per chunk
    for sg in seg_groups:
        w0 = segs[sg[0]][0]
        w1 = segs[sg[-1]][-1] + 1
        chunks.append((w0, w1 - w0))
    assert all(nw * P * NCORES < 32768 for _, nw in chunks)
    return dict(N=n, NLOC=nloc, NPAD=npad, NWIN=nwin, segs=segs,
                chunks=chunks, seg_groups=seg_groups)


def _build_perm(degs_total, npad):
    """Balanced node -> slot permutation. Snake-deal nodes (sorted by total
    degree desc) across all (core, window) bins so per-window edge counts are
    near-equal across cores."""
    n = len(degs_total)
    nloc = npad // NCORES
    nwin_total = npad // P
    tot = np.zeros(npad, np.int64)
    tot[:n] = degs_total
    order = np.argsort(-tot, kind="stable")
    slot_of = np.empty(npad, np.int64)
    counts = np.zeros(nwin_total, np.int64)
    i = np.arange(npad)
    rnd, pos = np.divmod(i, nwin_total)
    w = np.where(rnd % 2 == 0, pos, nwin_total - 1 - pos)
    core = w % NCORES
    j = w // NCORES
    for idx in range(npad):
        g = order[idx]
        ww = w[idx]
        slot_of[g] = core[idx] * nloc + j[idx] * P + counts[ww]
        counts[ww] += 1
    return slot_of


def _pack_idx(idx_flat):
    """[L] int16 -> wrapped [128, L//16] layout (16-partition wrap, replicated)."""
    L = len(idx_flat)
    assert L % 16 == 0
    base = idx_flat.reshape(L // 16, 16).T  # [16, L/16]
    return np.ascontiguousarray(np.tile(base, (8, 1))).astype(np.int16)


def _build_streams(plan, s_slot, d_slot, wgt):
    """Build per-core gather/one-hot streams for one relation.

    Edges are grouped by (dst core, dst window, src chunk); each group is
    padded to whole 128-edge tiles. Gather indices address the per-chunk
    AllGathered table [NCORES * chunk_wins * P, H].

    Stream tile order: for seg in segs: for k in chunks: for j in seg.
    """
    NLOC, NWIN = plan["NLOC"], plan["NWIN"]
    chunks = plan["chunks"]
    K = len(chunks)
    core = d_slot // NLOC
    j = (d_slot % NLOC) // P
    off = (d_slot % P).astype(np.float32)

    s_core = s_slot // NLOC
    s_loc = s_slot % NLOC
    s_win = s_loc // P
    win_chunk = np.zeros(NWIN, np.int64)
    win_local = np.zeros(NWIN, np.int64)
    for k, (w0, nw) in enumerate(chunks):
        win_chunk[w0:w0 + nw] = k
        win_local[w0:w0 + nw] = np.arange(nw)
    part = win_chunk[s_win]
    crows = np.array([nw * P for _, nw in chunks])
    idx_vals = s_core * crows[part] + win_local[s_win] * P + (s_loc % P)

    key = (core * NWIN + j) * K + part
    ngroups = NCORES * NWIN * K
    cnt = np.bincount(key, minlength=ngroups).reshape(NCORES, NWIN, K)
    T = np.ceil(cnt / P).astype(np.int64).max(axis=0)  # [NWIN, K]
    order = np.argsort(key, kind="stable")
    gstart = np.zeros(ngroups + 1, np.int64)
    np.cumsum(np.bincount(key, minlength=ngroups), out=gstart[1:])

    ntiles = int(T.sum())
    tile_off = {}
    tpos = 0
    for seg in plan["segs"]:
        for k in range(K):
            for jj in seg:
                tile_off[(jj, k)] = tpos
                tpos += int(T[jj, k])
    assert tpos == ntiles

    # sort edges within each group by source index (HBM row locality)
    order = order[np.lexsort((idx_vals[order], key[order]))]
    per_core = []
    for c in range(NCORES):
        idx_c = np.zeros(ntiles * P, np.int64)
        dq_c = np.full(ntiles * P, -1.0, np.float32)
        wq_c = np.zeros(ntiles * P, np.float32)
        for jj in range(NWIN):
            for k in range(K):
                g = (c * NWIN + jj) * K + k
                e = order[gstart[g]:gstart[g + 1]]
                L = len(e)
                if L == 0:
                    continue
                s0 = tile_off[(jj, k)] * P
                idx_c[s0:s0 + L] = idx_vals[e]
                dq_c[s0:s0 + L] = off[e]
                wq_c[s0:s0 + L] = wgt[e]
        per_core.append(dict(
            idx=_pack_idx(idx_c.astype(np.int16)),
            dq=np.ascontiguousarray(dq_c.reshape(ntiles, P).T),
            wq=np.ascontiguousarray(wq_c.reshape(ntiles, P).T),
        ))
    return T, ntiles, per_core


def preprocess(inputs):
    x = np.asarray(inputs["x"], np.float32)
    n = x.shape[0]
    plan = _make_plan(n)
    NLOC, NPAD = plan["NLOC"], plan["NPAD"]

    srcs, dsts, degs = [], [], []
    for r in range(R):
        s = np.asarray(inputs[f"src{r}"]).astype(np.int64)
        d = np.asarray(inputs[f"dst{r}"]).astype(np.int64)
        srcs.append(s)
        dsts.append(d)
        degs.append(np.bincount(d, minlength=n).astype(np.float64))
    perm = _build_perm(sum(degs)[:n].astype(np.int64), NPAD)  # global -> slot

    meta = dict(N=n, NLOC=NLOC, NPAD=NPAD, NWIN=plan["NWIN"],
                segs=tuple(tuple(s) for s in plan["segs"]),
                chunks=tuple(tuple(c) for c in plan["chunks"]),
                seg_groups=tuple(tuple(g) for g in plan["seg_groups"]))
    Ts, ntiles_l, streams = [], [], []
    for r in range(R):
        dinv = 1.0 / np.sqrt(np.maximum(degs[r], 1.0))
        wgt = (dinv[srcs[r]] * dinv[dsts[r]]).astype(np.float32)
        T, ntiles, per_core = _build_streams(
            plan, perm[srcs[r]], perm[dsts[r]], wgt)
        Ts.append(tuple(tuple(int(v) for v in row) for row in T))
        ntiles_l.append(ntiles)
        streams.append(per_core)
    meta["T"] = tuple(Ts)
    meta["ntiles"] = tuple(ntiles_l)

    x_slots = np.zeros((NPAD, IN_FEATS), np.float32)
    x_slots[perm[:n]] = x
    in_maps = []
    weight_names = []
    for r in range(R):
        weight_names += [f"W1_{r}", f"b1_{r}", f"W2_{r}", f"b2_{r}"]
    weight_names += ["W3", "b3"]
    for c in range(NCORES):
        m = {"xT": np.ascontiguousarray(
            x_slots[c * NLOC:(c + 1) * NLOC].T)}
        for name in weight_names:
            m[name] = np.asarray(inputs[name], np.float32)
        for r in range(R):
            m[f"idx{r}"] = streams[r][c]["idx"]
            m[f"dq{r}"] = streams[r][c]["dq"]
            m[f"wq{r}"] = streams[r][c]["wq"]
        in_maps.append(m)
    return meta, in_maps, perm


# ----------------------------------------------------------------------------
# Device program
# ----------------------------------------------------------------------------

def build_program(meta):
    NLOC, NPAD, NWIN = meta["NLOC"], meta["NPAD"], meta["NWIN"]
    segs = [list(s) for s in meta["segs"]]
    chunks = [tuple(c) for c in meta["chunks"]]
    seg_groups = [list(g) for g in meta["seg_groups"]]
    K = len(chunks)
    Ts = [np.array(t, np.int64) for t in meta["T"]]
    ntiles = meta["ntiles"]

    # max tiles in one gather call (seg x chunk), for V pool sizing
    maxcall = 1
    for r in range(R):
        for seg in segs:
            for k in range(K):
                maxcall = max(maxcall, int(Ts[r][seg, k].sum()))
    max_ntiles = max(ntiles)

    nc = bacc.Bacc("TRN2", target_bir_lowering=False, debug=False,
                   num_devices=NCORES)

    xT_d = nc.dram_tensor("xT", [IN_FEATS, NLOC], F32, kind="ExternalInput").ap()
    Wd = {}
    for r in range(R):
        Wd[f"W1_{r}"] = nc.dram_tensor(f"W1_{r}", [IN_FEATS, H], F32, kind="ExternalInput").ap()
        Wd[f"b1_{r}"] = nc.dram_tensor(f"b1_{r}", [H], F32, kind="ExternalInput").ap()
        Wd[f"W2_{r}"] = nc.dram_tensor(f"W2_{r}", [H, H], F32, kind="ExternalInput").ap()
        Wd[f"b2_{r}"] = nc.dram_tensor(f"b2_{r}", [H], F32, kind="ExternalInput").ap()
    W3_d = nc.dram_tensor("W3", [KORD * H, H], F32, kind="ExternalInput").ap()
    b3_d = nc.dram_tensor("b3", [H], F32, kind="ExternalInput").ap()
    idx_d, dq_d, wq_d = [], [], []
    for r in range(R):
        idx_d.append(nc.dram_tensor(f"idx{r}", [P, ntiles[r] * 8], I16, kind="ExternalInput").ap())
        dq_d.append(nc.dram_tensor(f"dq{r}", [P, ntiles[r]], F32, kind="ExternalInput").ap())
        wq_d.append(nc.dram_tensor(f"wq{r}", [P, ntiles[r]], F32, kind="ExternalInput").ap())
    out_d = nc.dram_tensor("out", [P, NLOC], F32, kind="ExternalOutput").ap()

    # internal DRAM: per-chunk AG inputs + shared tables, for h and T1
    aghin = [[nc.dram_tensor(f"aghin{r}_{k}", [nw * P, H], F16)
              for k, (w0, nw) in enumerate(chunks)] for r in range(R)]
    htab = [[nc.dram_tensor(f"htab{r}_{k}", [NCORES * nw * P, H], F16,
                            addr_space="Shared")
             for k, (w0, nw) in enumerate(chunks)] for r in range(R)]
    agtin = [[nc.dram_tensor(f"agtin{r}_{k}", [nw * P, H], F16)
              for k, (w0, nw) in enumerate(chunks)] for r in range(R)]
    ttab = [[nc.dram_tensor(f"ttab{r}_{k}", [NCORES * nw * P, H], F16,
                            addr_space="Shared")
             for k, (w0, nw) in enumerate(chunks)] for r in range(R)]

    with tile.TileContext(nc) as tc, ExitStack() as ctx:
        consts = ctx.enter_context(tc.tile_pool(name="consts", bufs=1))
        wtmp_p = ctx.enter_context(tc.tile_pool(name="wtmp", bufs=2))
        ht_p = ctx.enter_context(tc.tile_pool(name="ht", bufs=3))
        t1_p = ctx.enter_context(tc.tile_pool(name="t1", bufs=2))
        t2_p = ctx.enter_context(tc.tile_pool(name="t2", bufs=1))
        nm_p = ctx.enter_context(tc.tile_pool(name="nm", bufs=1))
        idx_p = ctx.enter_context(tc.tile_pool(name="idxp", bufs=2))
        dq_p = ctx.enter_context(tc.tile_pool(name="dqp", bufs=4))
        v_p = ctx.enter_context(tc.tile_pool(name="vp", bufs=2))
        m_p = ctx.enter_context(tc.tile_pool(name="mp", bufs=6))
        h1_p = ctx.enter_context(tc.tile_pool(name="h1p", bufs=3))
        oc_p = ctx.enter_context(tc.tile_pool(name="ocp", bufs=2))
        pp_big = ctx.enter_context(tc.tile_pool(name="ppbig", bufs=3, space="PSUM"))
        pp_hop = ctx.enter_context(tc.tile_pool(name="pphop", bufs=3, space="PSUM"))
        pp_tr = ctx.enter_context(tc.tile_pool(name="pptr", bufs=2, space="PSUM"))

        # ---- constants ----
        iota_i = consts.tile([P, P], I32, tag="iotai")
        nc.gpsimd.iota(iota_i[:], pattern=[[1, P]], base=0, channel_multiplier=0)
        iota_f = consts.tile([P, P], F16, tag="iotaf")
        nc.vector.tensor_copy(iota_f[:], iota_i[:])
        ident = consts.tile([P, P], F16, tag="ident")
        make_identity(nc, ident[:])
        zeroM = consts.tile([P, P], F16, tag="zerom")
        nc.vector.memset(zeroM[:], 0.0)

        cast_p = ctx.enter_context(tc.tile_pool(name="castp", bufs=2))

        def load_cast(dst, src_ap, n):
            c0 = 0
            while c0 < n:
                cw = min(512, n - c0)
                tmp = cast_p.tile([P, 512], F32, tag="cast")
                nc.sync.dma_start(out=tmp[:, 0:cw], in_=src_ap[:, c0:c0 + cw])
                nc.any.tensor_copy(dst[:, c0:c0 + cw], tmp[:, 0:cw])
                c0 += cw

        # x^T cast to fp16
        xT0 = consts.tile([P, NLOC], F16, tag="xt0")
        xT1 = consts.tile([P, NLOC], F16, tag="xt1")
        load_cast(xT0, xT_d[0:P, :], NLOC)
        load_cast(xT1, xT_d[P:2 * P, :], NLOC)

        # weights (cast fp16); biases fp32
        W1a, W1b, W2sb, b1c, b2c = [], [], [], [], []
        for r in range(R):
            wa = consts.tile([P, H], F16, tag=f"w1a{r}")
            wb = consts.tile([P, H], F16, tag=f"w1b{r}")
            w2 = consts.tile([P, H], F16, tag=f"w2{r}")
            load_cast(wa, Wd[f"W1_{r}"][0:P, :], H)
            load_cast(wb, Wd[f"W1_{r}"][P:2 * P, :], H)
            load_cast(w2, Wd[f"W2_{r}"][:, :], H)
            b1 = consts.tile([P, 1], F32, tag=f"b1{r}")
            b2 = consts.tile([P, 1], F32, tag=f"b2{r}")
            nc.sync.dma_start(out=b1[:], in_=Wd[f"b1_{r}"][:, None])
            nc.sync.dma_start(out=b2[:], in_=Wd[f"b2_{r}"][:, None])
            W1a.append(wa); W1b.append(wb); W2sb.append(w2)
            b1c.append(b1); b2c.append(b2)

        # W3 folded by Bernstein thetas: W3p_k = sum_j THETA[j,k] * W3_j
        w3s = []
        for jj in range(KORD):
            t = wtmp_p.tile([P, H], F32, tag=f"w3s{jj}")
            nc.sync.dma_start(out=t[:], in_=W3_d[jj * H:(jj + 1) * H, :])
            w3s.append(t)
        W3p = []
        for k in range(KORD):
            acc = wtmp_p.tile([P, H], F32, tag=f"w3acc{k}")
            nc.vector.tensor_scalar(out=acc[:], in0=w3s[0][:],
                                    scalar1=float(THETAS[0, k]), scalar2=None,
                                    op0=mybir.AluOpType.mult)
            for jj in range(1, KORD):
                t2t = wtmp_p.tile([P, H], F32, tag="w3mul")
                nc.vector.tensor_scalar(out=t2t[:], in0=w3s[jj][:],
                                        scalar1=float(THETAS[jj, k]), scalar2=None,
                                        op0=mybir.AluOpType.mult)
                nc.vector.tensor_tensor(out=acc[:], in0=acc[:], in1=t2t[:],
                                        op=mybir.AluOpType.add)
            wk = consts.tile([P, H], F16, tag=f"w3p{k}")
            nc.vector.tensor_copy(wk[:], acc[:])
            W3p.append(wk)
        b3x3 = consts.tile([P, 1], F32, tag="b3x3")
        nc.sync.dma_start(out=b3x3[:], in_=b3_d[:, None])
        nc.vector.tensor_scalar(out=b3x3[:], in0=b3x3[:], scalar1=3.0,
                                scalar2=None, op0=mybir.AluOpType.mult)

        out_acc = consts.tile([P, NLOC], F16, tag="outacc")

        def transpose_chunk(src_fm, nm_tile, k):
            """Transpose windows of chunk k from feat-major src into node-major
            nm_tile [P, nw*P]."""
            w0, nw = chunks[k]
            for j in range(nw):
                tp = pp_tr.tile([P, P], F16, space="PSUM", tag="tr")
                nc.tensor.transpose(out=tp[:],
                                    in_=src_fm[:, (w0 + j) * P:(w0 + j + 1) * P],
                                    identity=ident[:])
                nc.any.tensor_copy(nm_tile[:, j * P:(j + 1) * P], tp[:])

        def allgather_chunk(src_fm, ag_in_k, table_k, k):
            """Transpose chunk k of feat-major src, store to DRAM, AllGather."""
            w0, nw = chunks[k]
            nm_tile = nm_p.tile([P, max(nw for _, nw in chunks) * P], F16,
                                tag="nm")
            transpose_chunk(src_fm, nm_tile, k)
            nc.sync.dma_start(
                out=ag_in_k.ap().rearrange("(j p) f -> p j f", p=P),
                in_=nm_tile[:, 0:nw * P].rearrange("p (j f) -> p j f", f=H))
            nc.gpsimd.collective_compute(
                "AllGather", mybir.AluOpType.bypass,
                ins=[ag_in_k.ap()], outs=[table_k.ap()],
                replica_groups=[list(range(NCORES))])

        def hop(r, T, tables, prev_fm, next_fm, idx_sb, dq_sb, wq_sb,
                after_group=None):
            """next_fm = prev_fm - A_hat @ gathered(tables).

            after_group(ci) is called after all segs of seg_groups[ci] have
            their epilogue emitted (used to launch the next AG per chunk).
            """
            kmode = os.environ.get("KMODE", "full")
            if kmode == "nohop":
                nc.any.tensor_copy(next_fm[:], prev_fm[:])
                if after_group is not None:
                    for ci in range(K):
                        after_group(ci)
                return
            # stream column offset per (window, chunk), matching host layout
            stream_off = {}
            tpos = 0
            for seg in segs:
                for k in range(K):
                    for jj in seg:
                        stream_off[(jj, k)] = tpos
                        tpos += int(T[jj, k])
            icol = 0
            for ci, sg in enumerate(seg_groups):
                for si in sg:
                    seg = segs[si]
                    vbufs = {}
                    slot0 = {}
                    for k in range(K):
                        tcount = int(T[seg, k].sum())
                        if tcount == 0:
                            continue
                        vb = v_p.tile([P, maxcall * P], F16, tag="vbuf")
                        if kmode == "nogather":
                            nc.vector.memset(vb[:, 0:tcount * P], 0.0)
                        else:
                            nc.gpsimd.dma_gather(
                                out_ap=vb[:, 0:tcount * P].rearrange(
                                    "p (t e) -> p t e", e=P),
                                in_ap=tables[k].ap(),
                                idxs_ap=idx_sb[:, icol:icol + tcount * 8],
                                num_idxs=tcount * P,
                                num_idxs_reg=tcount * P,
                                elem_size=H,
                                single_packet=False,
                            )
                        icol += tcount * 8
                        vbufs[k] = vb
                        s = 0
                        for jj in seg:
                            slot0[(jj, k)] = s
                            s += int(T[jj, k])
                    j0 = seg[0]
                    bw = len(seg)
                    ps = pp_hop.tile([P, WIN_PER_SEG * P], F32, space="PSUM",
                                     tag="hop")
                    for jj in seg:
                        tot = int(T[jj].sum())
                        reg = ps[:, (jj - j0) * P:(jj - j0 + 1) * P]
                        if tot == 0:
                            nc.tensor.matmul(out=reg, lhsT=zeroM[:],
                                             rhs=zeroM[:], start=True, stop=True)
                            continue
                        kk = 0
                        for k in range(K):
                            for t in range(int(T[jj, k])):
                                col = stream_off[(jj, k)] + t
                                m = m_p.tile([P, P], F16, tag="onehot")
                                nc.any.tensor_scalar(
                                    out=m[:], in0=iota_f[:],
                                    scalar1=dq_sb[:, col:col + 1],
                                    scalar2=wq_sb[:, col:col + 1],
                                    op0=mybir.AluOpType.is_equal,
                                    op1=mybir.AluOpType.mult)
                                nc.tensor.matmul(
                                    out=reg,
                                    lhsT=vbufs[k][:, (slot0[(jj, k)] + t) * P:
                                                  (slot0[(jj, k)] + t + 1) * P],
                                    rhs=m[:],
                                    start=(kk == 0), stop=(kk == tot - 1))
                                kk += 1
                    nc.any.tensor_tensor(
                        out=next_fm[:, j0 * P:(j0 + bw) * P],
                        in0=prev_fm[:, j0 * P:(j0 + bw) * P],
                        in1=ps[:, 0:bw * P],
                        op=mybir.AluOpType.subtract)
                if after_group is not None:
                    after_group(ci)

        # ---- relations (software-pipelined) ----
        # Phase A: all MLPs, with chunked AG of h as each chunk completes.
        hTs = []
        for r in range(R):
            hT = ht_p.tile([P, NLOC], F16, tag="ht")
            for k, (w0, nw) in enumerate(chunks):
                c0 = w0 * P
                cend = (w0 + nw) * P
                while c0 < cend:
                    cw = min(MLP_CHUNK, cend - c0)
                    ps1 = pp_big.tile([P, MLP_CHUNK], F32, space="PSUM", tag="big")
                    nc.tensor.matmul(out=ps1[:, 0:cw], lhsT=W1a[r][:],
                                     rhs=xT0[:, c0:c0 + cw], start=True, stop=False)
                    nc.tensor.matmul(out=ps1[:, 0:cw], lhsT=W1b[r][:],
                                     rhs=xT1[:, c0:c0 + cw], start=False, stop=True)
                    h1 = h1_p.tile([P, MLP_CHUNK], F16, tag="h1")
                    nc.scalar.activation(h1[:, 0:cw], ps1[:, 0:cw],
                                         mybir.ActivationFunctionType.Lrelu,
                                         bias=b1c[r][:], scale=1.0, alpha=0.01)
                    ps2 = pp_big.tile([P, MLP_CHUNK], F32, space="PSUM", tag="big")
                    nc.tensor.matmul(out=ps2[:, 0:cw], lhsT=W2sb[r][:],
                                     rhs=h1[:, 0:cw], start=True, stop=True)
                    nc.scalar.activation(hT[:, c0:c0 + cw], ps2[:, 0:cw],
                                         mybir.ActivationFunctionType.Lrelu,
                                         bias=b2c[r][:], scale=1.0, alpha=0.01)
                    c0 += cw
                allgather_chunk(hT, aghin[r][k], htab[r][k], k)
            hTs.append(hT)

        # Phase B: hops + projection, interleaved across relations so each
        # AllGather transfer hides behind a full hop of another relation.
        streams_sb = {}

        def load_streams(r):
            idx_sb = idx_p.tile([P, max_ntiles * 8], I16, tag="idx")
            nc.sync.dma_start(out=idx_sb[:, 0:ntiles[r] * 8], in_=idx_d[r][:])
            dq_sb = dq_p.tile([P, max_ntiles], F32, tag="dq")
            wq_sb = dq_p.tile([P, max_ntiles], F32, tag="wq")
            nc.sync.dma_start(out=dq_sb[:, 0:ntiles[r]], in_=dq_d[r][:])
            nc.sync.dma_start(out=wq_sb[:, 0:ntiles[r]], in_=wq_d[r][:])
            streams_sb[r] = (idx_sb, dq_sb, wq_sb)

        T1s, T2s = {}, {}

        def hop1(r):
            load_streams(r)
            idx_sb, dq_sb, wq_sb = streams_sb[r]
            T1 = t1_p.tile([P, NLOC], F16, tag="t1")
            T1s[r] = T1

            def ag_t1(ci):
                allgather_chunk(T1, agtin[r][ci], ttab[r][ci], ci)

            hop(r, Ts[r], htab[r], hTs[r], T1, idx_sb, dq_sb, wq_sb,
                after_group=ag_t1)

        def hop2(r):
            idx_sb, dq_sb, wq_sb = streams_sb[r]
            T2 = t2_p.tile([P, NLOC], F16, tag="t2")
            T2s[r] = T2
            hop(r, Ts[r], ttab[r], T1s[r], T2, idx_sb, dq_sb, wq_sb)

        def proj(r):
            hT, T1, T2 = hTs[r], T1s[r], T2s[r]
            for c0 in range(0, NLOC, MLP_CHUNK):
                cw = min(MLP_CHUNK, NLOC - c0)
                psf = pp_big.tile([P, MLP_CHUNK], F32, space="PSUM", tag="big")
                nc.tensor.matmul(out=psf[:, 0:cw], lhsT=W3p[0][:],
                                 rhs=hT[:, c0:c0 + cw], start=True, stop=False)
                nc.tensor.matmul(out=psf[:, 0:cw], lhsT=W3p[1][:],
                                 rhs=T1[:, c0:c0 + cw], start=False, stop=False)
                nc.tensor.matmul(out=psf[:, 0:cw], lhsT=W3p[2][:],
                                 rhs=T2[:, c0:c0 + cw], start=False, stop=True)
                if r == 0:
                    nc.any.tensor_copy(out_acc[:, c0:c0 + cw], psf[:, 0:cw])
                else:
                    nc.any.tensor_tensor(out=out_acc[:, c0:c0 + cw],
                                         in0=out_acc[:, c0:c0 + cw],
                                         in1=psf[:, 0:cw],
                                         op=mybir.AluOpType.add)

        hop1(0)
        hop1(1)
        hop2(0)
        proj(0)
        hop1(2)
        hop2(1)
        proj(1)
        hop2(2)
        proj(2)

        # ---- output: leaky(out_acc + 3*b3), feat-major ----
        for c0 in range(0, NLOC, MLP_CHUNK):
            cw = min(MLP_CHUNK, NLOC - c0)
            oc = oc_p.tile([P, MLP_CHUNK], F32, tag="oc")
            nc.scalar.activation(oc[:, 0:cw], out_acc[:, c0:c0 + cw],
                                 mybir.ActivationFunctionType.Lrelu,
                                 bias=b3x3[:], scale=1.0, alpha=0.01)
            nc.sync.dma_start(out=out_d[:, c0:c0 + cw], in_=oc[:, 0:cw])

    nc.compile()
    return nc


# ----------------------------------------------------------------------------
# Entry point
# ----------------------------------------------------------------------------

_prog_cache = {}


def kernel(**inputs):
    meta, in_maps, perm = preprocess(inputs)
    key = repr((meta["N"], meta["NLOC"], meta["T"], meta["ntiles"], NCHUNK))
    if key not in _prog_cache:
        _prog_cache[key] = build_program(meta)
    nc = _prog_cache[key]
    res = run_bass_kernel_spmd(nc, in_maps, list(range(NCORES)))
    outs = [res.results[c]["out"] for c in range(NCORES)]  # [P, NLOC] each
    out_slots = np.concatenate(outs, axis=1).T             # [NPAD, H]
    n = meta["N"]
    return np.ascontiguousarray(out_slots[perm[:n]]).astype(np.float32)


# revision 9
# speedup vs baseline: 1.4962x; 1.4556x over previous
"""Trainium2 Bass kernel for nn_CombinedModel (3-relation GNN with Bernstein
polynomial message passing).

Self-contained: takes full inputs, shards nodes across 8 NeuronCores,
runs a Bass/Tile SPMD program (MLP -> 2 hops of normalized-Laplacian
aggregation -> polynomial projection), gathers the full output.

The node table used by the hop gathers is AllGathered in K chunks so the
collective pipelines against MLP / gather / matmul compute instead of
serializing in front of each hop.
"""
import math
import os
from contextlib import ExitStack

import numpy as np

import concourse.bacc as bacc
import concourse.tile as tile
from concourse import mybir
from concourse.bass_utils import run_bass_kernel_spmd
from concourse.masks import make_identity

F16, F32 = mybir.dt.float16, mybir.dt.float32
I16, I32 = mybir.dt.int16, mybir.dt.int32

NCORES = 8
P = 128
H = 128
IN_FEATS = 256
R = 3
D_ORDER = 2
KORD = D_ORDER + 1
WIN_PER_SEG = 4     # windows per gather segment == windows per PSUM bank
MLP_CHUNK = 512
NCHUNK = int(os.environ.get("KCHUNKS", "2"))   # table / AllGather chunks


def _bernstein_thetas(d):
    thetas = []
    for i in range(d + 1):
        a = np.zeros(i + 1)
        a[i] = 0.5 ** i
        b = np.array([math.comb(d - i, j) * (-0.5) ** j for j in range(d - i + 1)])
        scale = math.factorial(d + 1) / (math.factorial(i) * math.factorial(d - i))
        thetas.append((np.convolve(a, b) * scale).astype(np.float32))
    return np.stack(thetas)  # [d+1, d+1]


THETAS = _bernstein_thetas(D_ORDER)


# ----------------------------------------------------------------------------
# Host-side preprocessing
# ----------------------------------------------------------------------------

def _make_plan(n):
    nloc = ((n + NCORES * P - 1) // (NCORES * P)) * P
    npad = nloc * NCORES
    nwin = nloc // P
    segs = [list(range(s, min(s + WIN_PER_SEG, nwin)))
            for s in range(0, nwin, WIN_PER_SEG)]
    # chunks: K groups of consecutive segs, window counts as equal as possible
    nseg = len(segs)
    bounds = [round(i * nseg / NCHUNK) for i in range(NCHUNK + 1)]
    seg_groups = [list(range(bounds[i], bounds[i + 1])) for i in range(NCHUNK)]
    chunks = []  # (win_start, win_count) per chunk
    for sg in seg_groups:
        w0 = segs[sg[0]][0]
        w1 = segs[sg[-1]][-1] + 1
        chunks.append((w0, w1 - w0))
    assert all(nw * P * NCORES < 32768 for _, nw in chunks)
    return dict(N=n, NLOC=nloc, NPAD=npad, NWIN=nwin, segs=segs,
                chunks=chunks, seg_groups=seg_groups)


def _build_perm(degs_total, npad):
    """Balanced node -> slot permutation. Snake-deal nodes (sorted by total
    degree desc) across all (core, window) bins so per-window edge counts are
    near-equal across cores."""
    n = len(degs_total)
    nloc = npad // NCORES
    nwin_total = npad // P
    tot = np.zeros(npad, np.int64)
    tot[:n] = degs_total
    order = np.argsort(-tot, kind="stable")
    slot_of = np.empty(npad, np.int64)
    counts = np.zeros(nwin_total, np.int64)
    i = np.arange(npad)
    rnd, pos = np.divmod(i, nwin_total)
    w = np.where(rnd % 2 == 0, pos, nwin_total - 1 - pos)
    core = w % NCORES
    j = w // NCORES
    for idx in range(npad):
        g = order[idx]
        ww = w[idx]
        slot_of[g] = core[idx] * nloc + j[idx] * P + counts[ww]
        counts[ww] += 1
    return slot_of


def _pack_idx(idx_flat):
    """[L] int16 -> wrapped [128, L//16] layout (16-partition wrap, replicated)."""
    L = len(idx_flat)
    assert L % 16 == 0
    base = idx_flat.reshape(L // 16, 16).T  # [16, L/16]
    return np.ascontiguousarray(np.tile(base, (8, 1))).astype(np.int16)


def _build_streams(plan, s_slot, d_slot, wgt):
    """Build per-core gather/one-hot streams for one relation.

    Edges are grouped by (dst core, dst window, src chunk); each group is
    padded to whole 128-edge tiles. Gather indices address the per-chunk
    AllGathered table [NCORES * chunk_wins * P, H].

    Stream tile order: for seg in segs: for k in chunks: for j in seg.
    """
    NLOC, NWIN = plan["NLOC"], plan["NWIN"]
    chunks = plan["chunks"]
    K = len(chunks)
    core = d_slot // NLOC
    j = (d_slot % NLOC) // P
    off = (d_slot % P).astype(np.float32)

    s_core = s_slot // NLOC
    s_loc = s_slot % NLOC
    s_win = s_loc // P
    win_chunk = np.zeros(NWIN, np.int64)
    win_local = np.zeros(NWIN, np.int64)
    for k, (w0, nw) in enumerate(chunks):
        win_chunk[w0:w0 + nw] = k
        win_local[w0:w0 + nw] = np.arange(nw)
    part = win_chunk[s_win]
    crows = np.array([nw * P for _, nw in chunks])
    idx_vals = s_core * crows[part] + win_local[s_win] * P + (s_loc % P)

    key = (core * NWIN + j) * K + part
    ngroups = NCORES * NWIN * K
    cnt = np.bincount(key, minlength=ngroups).reshape(NCORES, NWIN, K)
    T = np.ceil(cnt / P).astype(np.int64).max(axis=0)  # [NWIN, K]
    order = np.argsort(key, kind="stable")
    gstart = np.zeros(ngroups + 1, np.int64)
    np.cumsum(np.bincount(key, minlength=ngroups), out=gstart[1:])

    ntiles = int(T.sum())
    tile_off = {}
    tpos = 0
    for seg in plan["segs"]:
        for k in range(K):
            for jj in seg:
                tile_off[(jj, k)] = tpos
                tpos += int(T[jj, k])
    assert tpos == ntiles

    # sort edges within each group by source index (HBM row locality)
    order = order[np.lexsort((idx_vals[order], key[order]))]
    per_core = []
    for c in range(NCORES):
        idx_c = np.zeros(ntiles * P, np.int64)
        dq_c = np.full(ntiles * P, -1.0, np.float32)
        wq_c = np.zeros(ntiles * P, np.float32)
        for jj in range(NWIN):
            for k in range(K):
                g = (c * NWIN + jj) * K + k
                e = order[gstart[g]:gstart[g + 1]]
                L = len(e)
                if L == 0:
                    continue
                s0 = tile_off[(jj, k)] * P
                idx_c[s0:s0 + L] = idx_vals[e]
                dq_c[s0:s0 + L] = off[e]
                wq_c[s0:s0 + L] = wgt[e]
        per_core.append(dict(
            idx=_pack_idx(idx_c.astype(np.int16)),
            dq=np.ascontiguousarray(dq_c.reshape(ntiles, P).T),
            wq=np.ascontiguousarray(wq_c.reshape(ntiles, P).T),
        ))
    return T, ntiles, per_core


def preprocess(inputs):
    x = np.asarray(inputs["x"], np.float32)
    n = x.shape[0]
    plan = _make_plan(n)
    NLOC, NPAD = plan["NLOC"], plan["NPAD"]

    srcs, dsts, degs = [], [], []
    for r in range(R):
        s = np.asarray(inputs[f"src{r}"]).astype(np.int64)
        d = np.asarray(inputs[f"dst{r}"]).astype(np.int64)
        srcs.append(s)
        dsts.append(d)
        degs.append(np.bincount(d, minlength=n).astype(np.float64))
    perm = _build_perm(sum(degs)[:n].astype(np.int64), NPAD)  # global -> slot

    meta = dict(N=n, NLOC=NLOC, NPAD=NPAD, NWIN=plan["NWIN"],
                segs=tuple(tuple(s) for s in plan["segs"]),
                chunks=tuple(tuple(c) for c in plan["chunks"]),
                seg_groups=tuple(tuple(g) for g in plan["seg_groups"]))
    Ts, ntiles_l, streams = [], [], []
    for r in range(R):
        dinv = 1.0 / np.sqrt(np.maximum(degs[r], 1.0))
        wgt = (dinv[srcs[r]] * dinv[dsts[r]]).astype(np.float32)
        T, ntiles, per_core = _build_streams(
            plan, perm[srcs[r]], perm[dsts[r]], wgt)
        Ts.append(tuple(tuple(int(v) for v in row) for row in T))
        ntiles_l.append(ntiles)
        streams.append(per_core)
    meta["T"] = tuple(Ts)
    meta["ntiles"] = tuple(ntiles_l)

    x_slots = np.zeros((NPAD, IN_FEATS), np.float32)
    x_slots[perm[:n]] = x
    in_maps = []
    weight_names = []
    for r in range(R):
        weight_names += [f"W1_{r}", f"b1_{r}", f"W2_{r}", f"b2_{r}"]
    weight_names += ["W3", "b3"]
    for c in range(NCORES):
        m = {"xT": np.ascontiguousarray(
            x_slots[c * NLOC:(c + 1) * NLOC].T)}
        for name in weight_names:
            m[name] = np.asarray(inputs[name], np.float32)
        for r in range(R):
            m[f"idx{r}"] = streams[r][c]["idx"]
            m[f"dq{r}"] = streams[r][c]["dq"]
            m[f"wq{r}"] = streams[r][c]["wq"]
        in_maps.append(m)
    return meta, in_maps, perm


# ----------------------------------------------------------------------------
# Device program
# ----------------------------------------------------------------------------

def build_program(meta):
    NLOC, NPAD, NWIN = meta["NLOC"], meta["NPAD"], meta["NWIN"]
    segs = [list(s) for s in meta["segs"]]
    chunks = [tuple(c) for c in meta["chunks"]]
    seg_groups = [list(g) for g in meta["seg_groups"]]
    K = len(chunks)
    Ts = [np.array(t, np.int64) for t in meta["T"]]
    ntiles = meta["ntiles"]

    # max tiles in one gather call (seg x chunk), for V pool sizing
    maxcall = 1
    for r in range(R):
        for seg in segs:
            for k in range(K):
                maxcall = max(maxcall, int(Ts[r][seg, k].sum()))
    max_ntiles = max(ntiles)

    NQ = int(os.environ.get("KQ", "4"))
    nc = bacc.Bacc("TRN2", target_bir_lowering=False, debug=False,
                   num_devices=NCORES, num_swdge_queues=NQ)

    xT_d = nc.dram_tensor("xT", [IN_FEATS, NLOC], F32, kind="ExternalInput").ap()
    Wd = {}
    for r in range(R):
        Wd[f"W1_{r}"] = nc.dram_tensor(f"W1_{r}", [IN_FEATS, H], F32, kind="ExternalInput").ap()
        Wd[f"b1_{r}"] = nc.dram_tensor(f"b1_{r}", [H], F32, kind="ExternalInput").ap()
        Wd[f"W2_{r}"] = nc.dram_tensor(f"W2_{r}", [H, H], F32, kind="ExternalInput").ap()
        Wd[f"b2_{r}"] = nc.dram_tensor(f"b2_{r}", [H], F32, kind="ExternalInput").ap()
    W3_d = nc.dram_tensor("W3", [KORD * H, H], F32, kind="ExternalInput").ap()
    b3_d = nc.dram_tensor("b3", [H], F32, kind="ExternalInput").ap()
    idx_d, dq_d, wq_d = [], [], []
    for r in range(R):
        idx_d.append(nc.dram_tensor(f"idx{r}", [P, ntiles[r] * 8], I16, kind="ExternalInput").ap())
        dq_d.append(nc.dram_tensor(f"dq{r}", [P, ntiles[r]], F32, kind="ExternalInput").ap())
        wq_d.append(nc.dram_tensor(f"wq{r}", [P, ntiles[r]], F32, kind="ExternalInput").ap())
    out_d = nc.dram_tensor("out", [P, NLOC], F32, kind="ExternalOutput").ap()

    # internal DRAM: per-chunk AG inputs + shared tables, for h and T1
    aghin = [[nc.dram_tensor(f"aghin{r}_{k}", [nw * P, H], F16)
              for k, (w0, nw) in enumerate(chunks)] for r in range(R)]
    htab = [[nc.dram_tensor(f"htab{r}_{k}", [NCORES * nw * P, H], F16,
                            addr_space="Shared")
             for k, (w0, nw) in enumerate(chunks)] for r in range(R)]
    agtin = [[nc.dram_tensor(f"agtin{r}_{k}", [nw * P, H], F16)
              for k, (w0, nw) in enumerate(chunks)] for r in range(R)]
    ttab = [[nc.dram_tensor(f"ttab{r}_{k}", [NCORES * nw * P, H], F16,
                            addr_space="Shared")
             for k, (w0, nw) in enumerate(chunks)] for r in range(R)]

    with tile.TileContext(nc) as tc, ExitStack() as ctx:
        consts = ctx.enter_context(tc.tile_pool(name="consts", bufs=1))
        wtmp_p = ctx.enter_context(tc.tile_pool(name="wtmp", bufs=2))
        ht_p = ctx.enter_context(tc.tile_pool(name="ht", bufs=3))
        t1_p = ctx.enter_context(tc.tile_pool(name="t1", bufs=2))
        t2_p = ctx.enter_context(tc.tile_pool(name="t2", bufs=1))
        nm_p = ctx.enter_context(tc.tile_pool(name="nm", bufs=1))
        idx_p = ctx.enter_context(tc.tile_pool(name="idxp", bufs=2))
        dq_p = ctx.enter_context(tc.tile_pool(name="dqp", bufs=4))
        v_p = ctx.enter_context(tc.tile_pool(name="vp", bufs=2))
        m_p = ctx.enter_context(tc.tile_pool(name="mp", bufs=6))
        h1_p = ctx.enter_context(tc.tile_pool(name="h1p", bufs=3))
        oc_p = ctx.enter_context(tc.tile_pool(name="ocp", bufs=2))
        pp_big = ctx.enter_context(tc.tile_pool(name="ppbig", bufs=3, space="PSUM"))
        pp_hop = ctx.enter_context(tc.tile_pool(name="pphop", bufs=3, space="PSUM"))
        pp_tr = ctx.enter_context(tc.tile_pool(name="pptr", bufs=2, space="PSUM"))

        # ---- constants ----
        iota_i = consts.tile([P, P], I32, tag="iotai")
        nc.gpsimd.iota(iota_i[:], pattern=[[1, P]], base=0, channel_multiplier=0)
        iota_f = consts.tile([P, P], F16, tag="iotaf")
        nc.vector.tensor_copy(iota_f[:], iota_i[:])
        ident = consts.tile([P, P], F16, tag="ident")
        make_identity(nc, ident[:])
        zeroM = consts.tile([P, P], F16, tag="zerom")
        nc.vector.memset(zeroM[:], 0.0)

        cast_p = ctx.enter_context(tc.tile_pool(name="castp", bufs=2))

        def load_cast(dst, src_ap, n):
            c0 = 0
            while c0 < n:
                cw = min(512, n - c0)
                tmp = cast_p.tile([P, 512], F32, tag="cast")
                nc.sync.dma_start(out=tmp[:, 0:cw], in_=src_ap[:, c0:c0 + cw])
                nc.any.tensor_copy(dst[:, c0:c0 + cw], tmp[:, 0:cw])
                c0 += cw

        # x^T cast to fp16
        xT0 = consts.tile([P, NLOC], F16, tag="xt0")
        xT1 = consts.tile([P, NLOC], F16, tag="xt1")
        load_cast(xT0, xT_d[0:P, :], NLOC)
        load_cast(xT1, xT_d[P:2 * P, :], NLOC)

        # weights (cast fp16); biases fp32
        W1a, W1b, W2sb, b1c, b2c = [], [], [], [], []
        for r in range(R):
            wa = consts.tile([P, H], F16, tag=f"w1a{r}")
            wb = consts.tile([P, H], F16, tag=f"w1b{r}")
            w2 = consts.tile([P, H], F16, tag=f"w2{r}")
            load_cast(wa, Wd[f"W1_{r}"][0:P, :], H)
            load_cast(wb, Wd[f"W1_{r}"][P:2 * P, :], H)
            load_cast(w2, Wd[f"W2_{r}"][:, :], H)
            b1 = consts.tile([P, 1], F32, tag=f"b1{r}")
            b2 = consts.tile([P, 1], F32, tag=f"b2{r}")
            nc.sync.dma_start(out=b1[:], in_=Wd[f"b1_{r}"][:, None])
            nc.sync.dma_start(out=b2[:], in_=Wd[f"b2_{r}"][:, None])
            W1a.append(wa); W1b.append(wb); W2sb.append(w2)
            b1c.append(b1); b2c.append(b2)

        # W3 folded by Bernstein thetas: W3p_k = sum_j THETA[j,k] * W3_j
        w3s = []
        for jj in range(KORD):
            t = wtmp_p.tile([P, H], F32, tag=f"w3s{jj}")
            nc.sync.dma_start(out=t[:], in_=W3_d[jj * H:(jj + 1) * H, :])
            w3s.append(t)
        W3p = []
        for k in range(KORD):
            acc = wtmp_p.tile([P, H], F32, tag=f"w3acc{k}")
            nc.vector.tensor_scalar(out=acc[:], in0=w3s[0][:],
                                    scalar1=float(THETAS[0, k]), scalar2=None,
                                    op0=mybir.AluOpType.mult)
            for jj in range(1, KORD):
                t2t = wtmp_p.tile([P, H], F32, tag="w3mul")
                nc.vector.tensor_scalar(out=t2t[:], in0=w3s[jj][:],
                                        scalar1=float(THETAS[jj, k]), scalar2=None,
                                        op0=mybir.AluOpType.mult)
                nc.vector.tensor_tensor(out=acc[:], in0=acc[:], in1=t2t[:],
                                        op=mybir.AluOpType.add)
            wk = consts.tile([P, H], F16, tag=f"w3p{k}")
            nc.vector.tensor_copy(wk[:], acc[:])
            W3p.append(wk)
        b3x3 = consts.tile([P, 1], F32, tag="b3x3")
        nc.sync.dma_start(out=b3x3[:], in_=b3_d[:, None])
        nc.vector.tensor_scalar(out=b3x3[:], in0=b3x3[:], scalar1=3.0,
                                scalar2=None, op0=mybir.AluOpType.mult)

        out_acc = consts.tile([P, NLOC], F16, tag="outacc")

        def transpose_chunk(src_fm, nm_tile, k):
            """Transpose windows of chunk k from feat-major src into node-major
            nm_tile [P, nw*P]."""
            w0, nw = chunks[k]
            for j in range(nw):
                tp = pp_tr.tile([P, P], F16, space="PSUM", tag="tr")
                nc.tensor.transpose(out=tp[:],
                                    in_=src_fm[:, (w0 + j) * P:(w0 + j + 1) * P],
                                    identity=ident[:])
                nc.any.tensor_copy(nm_tile[:, j * P:(j + 1) * P], tp[:])

        def allgather_chunk(src_fm, ag_in_k, table_k, k):
            """Transpose chunk k of feat-major src, store to DRAM, AllGather."""
            w0, nw = chunks[k]
            nm_tile = nm_p.tile([P, max(nw for _, nw in chunks) * P], F16,
                                tag="nm")
            transpose_chunk(src_fm, nm_tile, k)
            nc.sync.dma_start(
                out=ag_in_k.ap().rearrange("(j p) f -> p j f", p=P),
                in_=nm_tile[:, 0:nw * P].rearrange("p (j f) -> p j f", f=H))
            if os.environ.get("KSKIP_AG"):
                return
            nc.gpsimd.collective_compute(
                "AllGather", mybir.AluOpType.bypass,
                ins=[ag_in_k.ap()], outs=[table_k.ap()],
                replica_groups=[list(range(NCORES))])

        def hop(r, T, tables, prev_fm, next_fm, idx_sb, dq_sb, wq_sb,
                after_group=None):
            """next_fm = prev_fm - A_hat @ gathered(tables).

            after_group(ci) is called after all segs of seg_groups[ci] have
            their epilogue emitted (used to launch the next AG per chunk).
            """
            kmode = os.environ.get("KMODE", "full")
            if kmode == "nohop":
                nc.any.tensor_copy(next_fm[:], prev_fm[:])
                if after_group is not None:
                    for ci in range(K):
                        after_group(ci)
                return
            # stream column offset per (window, chunk), matching host layout
            stream_off = {}
            tpos = 0
            for seg in segs:
                for k in range(K):
                    for jj in seg:
                        stream_off[(jj, k)] = tpos
                        tpos += int(T[jj, k])
            single_pkt = bool(os.environ.get("KSP"))
            icol = 0
            for ci, sg in enumerate(seg_groups):
                for si in sg:
                    seg = segs[si]
                    vbufs = {}
                    slot0 = {}
                    for k in range(K):
                        tcount = int(T[seg, k].sum())
                        if tcount == 0:
                            continue
                        vb = v_p.tile([P, maxcall * P], F16, tag="vbuf")
                        if kmode == "nogather":
                            nc.vector.memset(vb[:, 0:tcount * P], 0.0)
                        else:
                            nc._gq = getattr(nc, "_gq", -1) + 1
                            nc.gpsimd.dma_gather(
                                out_ap=vb[:, 0:tcount * P].rearrange(
                                    "p (t e) -> p t e", e=P),
                                in_ap=tables[k].ap(),
                                idxs_ap=idx_sb[:, icol:icol + tcount * 8],
                                num_idxs=tcount * P,
                                num_idxs_reg=tcount * P,
                                elem_size=H,
                                single_packet=single_pkt,
                                queue_num=nc._gq % nc.num_swdge_queues,
                            )
                        icol += tcount * 8
                        vbufs[k] = vb
                        s = 0
                        for jj in seg:
                            slot0[(jj, k)] = s
                            s += int(T[jj, k])
                    j0 = seg[0]
                    bw = len(seg)
                    ps = pp_hop.tile([P, WIN_PER_SEG * P], F32, space="PSUM",
                                     tag="hop")
                    for jj in seg:
                        tot = int(T[jj].sum())
                        reg = ps[:, (jj - j0) * P:(jj - j0 + 1) * P]
                        if tot == 0:
                            nc.tensor.matmul(out=reg, lhsT=zeroM[:],
                                             rhs=zeroM[:], start=True, stop=True)
                            continue
                        kk = 0
                        for k in range(K):
                            for t in range(int(T[jj, k])):
                                col = stream_off[(jj, k)] + t
                                m = m_p.tile([P, P], F16, tag="onehot")
                                nc.any.tensor_scalar(
                                    out=m[:], in0=iota_f[:],
                                    scalar1=dq_sb[:, col:col + 1],
                                    scalar2=wq_sb[:, col:col + 1],
                                    op0=mybir.AluOpType.is_equal,
                                    op1=mybir.AluOpType.mult)
                                nc.tensor.matmul(
                                    out=reg,
                                    lhsT=vbufs[k][:, (slot0[(jj, k)] + t) * P:
                                                  (slot0[(jj, k)] + t + 1) * P],
                                    rhs=m[:],
                                    start=(kk == 0), stop=(kk == tot - 1))
                                kk += 1
                    nc.any.tensor_tensor(
                        out=next_fm[:, j0 * P:(j0 + bw) * P],
                        in0=prev_fm[:, j0 * P:(j0 + bw) * P],
                        in1=ps[:, 0:bw * P],
                        op=mybir.AluOpType.subtract)
                if after_group is not None:
                    after_group(ci)

        # ---- relations (software-pipelined) ----
        # Phase A: all MLPs, with chunked AG of h as each chunk completes.
        hTs = []
        for r in range(R):
            hT = ht_p.tile([P, NLOC], F16, tag="ht")
            for k, (w0, nw) in enumerate(chunks):
                c0 = w0 * P
                cend = (w0 + nw) * P
                while c0 < cend:
                    cw = min(MLP_CHUNK, cend - c0)
                    ps1 = pp_big.tile([P, MLP_CHUNK], F32, space="PSUM", tag="big")
                    nc.tensor.matmul(out=ps1[:, 0:cw], lhsT=W1a[r][:],
                                     rhs=xT0[:, c0:c0 + cw], start=True, stop=False)
                    nc.tensor.matmul(out=ps1[:, 0:cw], lhsT=W1b[r][:],
                                     rhs=xT1[:, c0:c0 + cw], start=False, stop=True)
                    h1 = h1_p.tile([P, MLP_CHUNK], F16, tag="h1")
                    nc.scalar.activation(h1[:, 0:cw], ps1[:, 0:cw],
                                         mybir.ActivationFunctionType.Lrelu,
                                         bias=b1c[r][:], scale=1.0, alpha=0.01)
                    ps2 = pp_big.tile([P, MLP_CHUNK], F32, space="PSUM", tag="big")
                    nc.tensor.matmul(out=ps2[:, 0:cw], lhsT=W2sb[r][:],
                                     rhs=h1[:, 0:cw], start=True, stop=True)
                    nc.scalar.activation(hT[:, c0:c0 + cw], ps2[:, 0:cw],
                                         mybir.ActivationFunctionType.Lrelu,
                                         bias=b2c[r][:], scale=1.0, alpha=0.01)
                    c0 += cw
                allgather_chunk(hT, aghin[r][k], htab[r][k], k)
            hTs.append(hT)

        # Phase B: hops + projection, interleaved across relations so each
        # AllGather transfer hides behind a full hop of another relation.
        streams_sb = {}

        def load_streams(r):
            idx_sb = idx_p.tile([P, max_ntiles * 8], I16, tag="idx")
            nc.sync.dma_start(out=idx_sb[:, 0:ntiles[r] * 8], in_=idx_d[r][:])
            dq_sb = dq_p.tile([P, max_ntiles], F32, tag="dq")
            wq_sb = dq_p.tile([P, max_ntiles], F32, tag="wq")
            nc.sync.dma_start(out=dq_sb[:, 0:ntiles[r]], in_=dq_d[r][:])
            nc.sync.dma_start(out=wq_sb[:, 0:ntiles[r]], in_=wq_d[r][:])
            streams_sb[r] = (idx_sb, dq_sb, wq_sb)

        T1s, T2s = {}, {}

        def hop1(r):
            load_streams(r)
            idx_sb, dq_sb, wq_sb = streams_sb[r]
            T1 = t1_p.tile([P, NLOC], F16, tag="t1")
            T1s[r] = T1

            def ag_t1(ci):
                allgather_chunk(T1, agtin[r][ci], ttab[r][ci], ci)

            hop(r, Ts[r], htab[r], hTs[r], T1, idx_sb, dq_sb, wq_sb,
                after_group=ag_t1)

        def hop2(r):
            idx_sb, dq_sb, wq_sb = streams_sb[r]
            T2 = t2_p.tile([P, NLOC], F16, tag="t2")
            T2s[r] = T2
            hop(r, Ts[r], ttab[r], T1s[r], T2, idx_sb, dq_sb, wq_sb)

        def proj(r):
            hT, T1, T2 = hTs[r], T1s[r], T2s[r]
            for c0 in range(0, NLOC, MLP_CHUNK):
                cw = min(MLP_CHUNK, NLOC - c0)
                psf = pp_big.tile([P, MLP_CHUNK], F32, space="PSUM", tag="big")
                nc.tensor.matmul(out=psf[:, 0:cw], lhsT=W3p[0][:],
                                 rhs=hT[:, c0:c0 + cw], start=True, stop=False)
                nc.tensor.matmul(out=psf[:, 0:cw], lhsT=W3p[1][:],
                                 rhs=T1[:, c0:c0 + cw], start=False, stop=False)
                nc.tensor.matmul(out=psf[:, 0:cw], lhsT=W3p[2][:],
                                 rhs=T2[:, c0:c0 + cw], start=False, stop=True)
                if r == 0:
                    nc.any.tensor_copy(out_acc[:, c0:c0 + cw], psf[:, 0:cw])
                else:
                    nc.any.tensor_tensor(out=out_acc[:, c0:c0 + cw],
                                         in0=out_acc[:, c0:c0 + cw],
                                         in1=psf[:, 0:cw],
                                         op=mybir.AluOpType.add)

        hop1(0)
        hop1(1)
        hop2(0)
        proj(0)
        hop1(2)
        hop2(1)
        proj(1)
        hop2(2)
        proj(2)

        # ---- output: leaky(out_acc + 3*b3), feat-major ----
        for c0 in range(0, NLOC, MLP_CHUNK):
            cw = min(MLP_CHUNK, NLOC - c0)
            oc = oc_p.tile([P, MLP_CHUNK], F32, tag="oc")
            nc.scalar.activation(oc[:, 0:cw], out_acc[:, c0:c0 + cw],
                                 mybir.ActivationFunctionType.Lrelu,
                                 bias=b3x3[:], scale=1.0, alpha=0.01)
            nc.sync.dma_start(out=out_d[:, c0:c0 + cw], in_=oc[:, 0:cw])

    nc.compile()
    return nc


# ----------------------------------------------------------------------------
# Entry point
# ----------------------------------------------------------------------------

_prog_cache = {}


def kernel(**inputs):
    meta, in_maps, perm = preprocess(inputs)
    key = repr((meta["N"], meta["NLOC"], meta["T"], meta["ntiles"], NCHUNK))
    if key not in _prog_cache:
        _prog_cache[key] = build_program(meta)
    nc = _prog_cache[key]
    res = run_bass_kernel_spmd(nc, in_maps, list(range(NCORES)))
    outs = [res.results[c]["out"] for c in range(NCORES)]  # [P, NLOC] each
    out_slots = np.concatenate(outs, axis=1).T             # [NPAD, H]
    n = meta["N"]
    return np.ascontiguousarray(out_slots[perm[:n]]).astype(np.float32)


# revision 12
# speedup vs baseline: 1.5200x; 1.0159x over previous
"""Trainium2 Bass kernel for nn_CombinedModel (3-relation GNN with Bernstein
polynomial message passing).

Self-contained: takes full inputs, shards nodes across 8 NeuronCores,
runs a Bass/Tile SPMD program (MLP -> 2 hops of normalized-Laplacian
aggregation -> polynomial projection), gathers the full output.

The node table used by the hop gathers is AllGathered in K chunks so the
collective pipelines against MLP / gather / matmul compute instead of
serializing in front of each hop.
"""
import math
import os
from contextlib import ExitStack

import numpy as np

import concourse.bacc as bacc
import concourse.tile as tile
from concourse import mybir
from concourse.bass_utils import run_bass_kernel_spmd
from concourse.masks import make_identity

F16, F32 = mybir.dt.float16, mybir.dt.float32
I16, I32 = mybir.dt.int16, mybir.dt.int32

NCORES = 8
P = 128
H = 128
IN_FEATS = 256
R = 3
D_ORDER = 2
KORD = D_ORDER + 1
WIN_PER_SEG = 4     # windows per gather segment == windows per PSUM bank
MLP_CHUNK = 512
NCHUNK = int(os.environ.get("KCHUNKS", "2"))   # table / AllGather chunks


def _bernstein_thetas(d):
    thetas = []
    for i in range(d + 1):
        a = np.zeros(i + 1)
        a[i] = 0.5 ** i
        b = np.array([math.comb(d - i, j) * (-0.5) ** j for j in range(d - i + 1)])
        scale = math.factorial(d + 1) / (math.factorial(i) * math.factorial(d - i))
        thetas.append((np.convolve(a, b) * scale).astype(np.float32))
    return np.stack(thetas)  # [d+1, d+1]


THETAS = _bernstein_thetas(D_ORDER)


# ----------------------------------------------------------------------------
# Host-side preprocessing
# ----------------------------------------------------------------------------

def _make_plan(n):
    nloc = ((n + NCORES * P - 1) // (NCORES * P)) * P
    npad = nloc * NCORES
    nwin = nloc // P
    segs = [list(range(s, min(s + WIN_PER_SEG, nwin)))
            for s in range(0, nwin, WIN_PER_SEG)]
    # chunks: K groups of consecutive segs, window counts as equal as possible
    nseg = len(segs)
    bounds = [round(i * nseg / NCHUNK) for i in range(NCHUNK + 1)]
    seg_groups = [list(range(bounds[i], bounds[i + 1])) for i in range(NCHUNK)]
    chunks = []  # (win_start, win_count) per chunk
    for sg in seg_groups:
        w0 = segs[sg[0]][0]
        w1 = segs[sg[-1]][-1] + 1
        chunks.append((w0, w1 - w0))
    assert all(nw * P * NCORES < 32768 for _, nw in chunks)
    return dict(N=n, NLOC=nloc, NPAD=npad, NWIN=nwin, segs=segs,
                chunks=chunks, seg_groups=seg_groups)


def _build_perm(degs_total, npad):
    """Balanced node -> slot permutation. Snake-deal nodes (sorted by total
    degree desc) across all (core, window) bins so per-window edge counts are
    near-equal across cores."""
    n = len(degs_total)
    nloc = npad // NCORES
    nwin_total = npad // P
    tot = np.zeros(npad, np.int64)
    tot[:n] = degs_total
    order = np.argsort(-tot, kind="stable")
    slot_of = np.empty(npad, np.int64)
    counts = np.zeros(nwin_total, np.int64)
    i = np.arange(npad)
    rnd, pos = np.divmod(i, nwin_total)
    w = np.where(rnd % 2 == 0, pos, nwin_total - 1 - pos)
    core = w % NCORES
    j = w // NCORES
    for idx in range(npad):
        g = order[idx]
        ww = w[idx]
        slot_of[g] = core[idx] * nloc + j[idx] * P + counts[ww]
        counts[ww] += 1
    return slot_of


def _pack_idx(idx_flat):
    """[L] int16 -> wrapped [128, L//16] layout (16-partition wrap, replicated)."""
    L = len(idx_flat)
    assert L % 16 == 0
    base = idx_flat.reshape(L // 16, 16).T  # [16, L/16]
    return np.ascontiguousarray(np.tile(base, (8, 1))).astype(np.int16)


def _build_streams(plan, s_slot, d_slot, wgt):
    """Build per-core gather/one-hot streams for one relation.

    Edges are grouped by (dst core, dst window, src chunk); each group is
    padded to whole 128-edge tiles. Gather indices address the per-chunk
    AllGathered table [NCORES * chunk_wins * P, H].

    Stream tile order: for seg in segs: for k in chunks: for j in seg.
    """
    NLOC, NWIN = plan["NLOC"], plan["NWIN"]
    chunks = plan["chunks"]
    K = len(chunks)
    core = d_slot // NLOC
    j = (d_slot % NLOC) // P
    off = (d_slot % P).astype(np.float32)

    s_core = s_slot // NLOC
    s_loc = s_slot % NLOC
    s_win = s_loc // P
    win_chunk = np.zeros(NWIN, np.int64)
    win_local = np.zeros(NWIN, np.int64)
    for k, (w0, nw) in enumerate(chunks):
        win_chunk[w0:w0 + nw] = k
        win_local[w0:w0 + nw] = np.arange(nw)
    part = win_chunk[s_win]
    crows = np.array([nw * P for _, nw in chunks])
    idx_vals = s_core * crows[part] + win_local[s_win] * P + (s_loc % P)

    key = (core * NWIN + j) * K + part
    ngroups = NCORES * NWIN * K
    cnt = np.bincount(key, minlength=ngroups).reshape(NCORES, NWIN, K)
    T = np.ceil(cnt / P).astype(np.int64).max(axis=0)  # [NWIN, K]
    order = np.argsort(key, kind="stable")
    gstart = np.zeros(ngroups + 1, np.int64)
    np.cumsum(np.bincount(key, minlength=ngroups), out=gstart[1:])

    ntiles = int(T.sum())
    tile_off = {}
    tpos = 0
    for seg in plan["segs"]:
        for k in range(K):
            for jj in seg:
                tile_off[(jj, k)] = tpos
                tpos += int(T[jj, k])
    assert tpos == ntiles

    # sort edges within each group by source index (HBM row locality)
    order = order[np.lexsort((idx_vals[order], key[order]))]
    per_core = []
    for c in range(NCORES):
        idx_c = np.zeros(ntiles * P, np.int64)
        dq_c = np.full(ntiles * P, -1.0, np.float32)
        wq_c = np.zeros(ntiles * P, np.float32)
        for jj in range(NWIN):
            for k in range(K):
                g = (c * NWIN + jj) * K + k
                e = order[gstart[g]:gstart[g + 1]]
                L = len(e)
                if L == 0:
                    continue
                s0 = tile_off[(jj, k)] * P
                idx_c[s0:s0 + L] = idx_vals[e]
                dq_c[s0:s0 + L] = off[e]
                wq_c[s0:s0 + L] = wgt[e]
        per_core.append(dict(
            idx=_pack_idx(idx_c.astype(np.int16)),
            dq=np.ascontiguousarray(dq_c.reshape(ntiles, P).T),
            wq=np.ascontiguousarray(wq_c.reshape(ntiles, P).T),
        ))
    return T, ntiles, per_core


def preprocess(inputs):
    x = np.asarray(inputs["x"], np.float32)
    n = x.shape[0]
    plan = _make_plan(n)
    NLOC, NPAD = plan["NLOC"], plan["NPAD"]

    srcs, dsts, degs = [], [], []
    for r in range(R):
        s = np.asarray(inputs[f"src{r}"]).astype(np.int64)
        d = np.asarray(inputs[f"dst{r}"]).astype(np.int64)
        srcs.append(s)
        dsts.append(d)
        degs.append(np.bincount(d, minlength=n).astype(np.float64))
    perm = _build_perm(sum(degs)[:n].astype(np.int64), NPAD)  # global -> slot

    meta = dict(N=n, NLOC=NLOC, NPAD=NPAD, NWIN=plan["NWIN"],
                segs=tuple(tuple(s) for s in plan["segs"]),
                chunks=tuple(tuple(c) for c in plan["chunks"]),
                seg_groups=tuple(tuple(g) for g in plan["seg_groups"]))
    Ts, ntiles_l, streams = [], [], []
    for r in range(R):
        dinv = 1.0 / np.sqrt(np.maximum(degs[r], 1.0))
        wgt = (dinv[srcs[r]] * dinv[dsts[r]]).astype(np.float32)
        T, ntiles, per_core = _build_streams(
            plan, perm[srcs[r]], perm[dsts[r]], wgt)
        Ts.append(tuple(tuple(int(v) for v in row) for row in T))
        ntiles_l.append(ntiles)
        streams.append(per_core)
    meta["T"] = tuple(Ts)
    meta["ntiles"] = tuple(ntiles_l)

    x_slots = np.zeros((NPAD, IN_FEATS), np.float32)
    x_slots[perm[:n]] = x
    in_maps = []
    weight_names = []
    for r in range(R):
        weight_names += [f"W1_{r}", f"b1_{r}", f"W2_{r}", f"b2_{r}"]
    weight_names += ["W3", "b3"]
    for c in range(NCORES):
        m = {"xT": np.ascontiguousarray(
            x_slots[c * NLOC:(c + 1) * NLOC].T)}
        for name in weight_names:
            m[name] = np.asarray(inputs[name], np.float32)
        for r in range(R):
            m[f"idx{r}"] = streams[r][c]["idx"]
            m[f"dq{r}"] = streams[r][c]["dq"]
            m[f"wq{r}"] = streams[r][c]["wq"]
        in_maps.append(m)
    return meta, in_maps, perm


# ----------------------------------------------------------------------------
# Device program
# ----------------------------------------------------------------------------

def build_program(meta):
    NLOC, NPAD, NWIN = meta["NLOC"], meta["NPAD"], meta["NWIN"]
    segs = [list(s) for s in meta["segs"]]
    chunks = [tuple(c) for c in meta["chunks"]]
    seg_groups = [list(g) for g in meta["seg_groups"]]
    K = len(chunks)
    Ts = [np.array(t, np.int64) for t in meta["T"]]
    ntiles = meta["ntiles"]

    # max tiles in one gather call (seg x chunk), for V pool sizing
    maxcall = 1
    for r in range(R):
        for seg in segs:
            for k in range(K):
                maxcall = max(maxcall, int(Ts[r][seg, k].sum()))
    max_ntiles = max(ntiles)

    NQ = int(os.environ.get("KQ", "4"))
    nc = bacc.Bacc("TRN2", target_bir_lowering=False, debug=False,
                   num_devices=NCORES, num_swdge_queues=NQ)

    xT_d = nc.dram_tensor("xT", [IN_FEATS, NLOC], F32, kind="ExternalInput").ap()
    Wd = {}
    for r in range(R):
        Wd[f"W1_{r}"] = nc.dram_tensor(f"W1_{r}", [IN_FEATS, H], F32, kind="ExternalInput").ap()
        Wd[f"b1_{r}"] = nc.dram_tensor(f"b1_{r}", [H], F32, kind="ExternalInput").ap()
        Wd[f"W2_{r}"] = nc.dram_tensor(f"W2_{r}", [H, H], F32, kind="ExternalInput").ap()
        Wd[f"b2_{r}"] = nc.dram_tensor(f"b2_{r}", [H], F32, kind="ExternalInput").ap()
    W3_d = nc.dram_tensor("W3", [KORD * H, H], F32, kind="ExternalInput").ap()
    b3_d = nc.dram_tensor("b3", [H], F32, kind="ExternalInput").ap()
    idx_d, dq_d, wq_d = [], [], []
    for r in range(R):
        idx_d.append(nc.dram_tensor(f"idx{r}", [P, ntiles[r] * 8], I16, kind="ExternalInput").ap())
        dq_d.append(nc.dram_tensor(f"dq{r}", [P, ntiles[r]], F32, kind="ExternalInput").ap())
        wq_d.append(nc.dram_tensor(f"wq{r}", [P, ntiles[r]], F32, kind="ExternalInput").ap())
    out_d = nc.dram_tensor("out", [P, NLOC], F32, kind="ExternalOutput").ap()

    # internal DRAM: per-chunk AG inputs + shared tables, for h and T1
    aghin = [[nc.dram_tensor(f"aghin{r}_{k}", [nw * P, H], F16)
              for k, (w0, nw) in enumerate(chunks)] for r in range(R)]
    htab = [[nc.dram_tensor(f"htab{r}_{k}", [NCORES * nw * P, H], F16,
                            addr_space="Shared")
             for k, (w0, nw) in enumerate(chunks)] for r in range(R)]
    agtin = [[nc.dram_tensor(f"agtin{r}_{k}", [nw * P, H], F16)
              for k, (w0, nw) in enumerate(chunks)] for r in range(R)]
    ttab = [[nc.dram_tensor(f"ttab{r}_{k}", [NCORES * nw * P, H], F16,
                            addr_space="Shared")
             for k, (w0, nw) in enumerate(chunks)] for r in range(R)]

    with tile.TileContext(nc) as tc, ExitStack() as ctx:
        consts = ctx.enter_context(tc.tile_pool(name="consts", bufs=1))
        wtmp_p = ctx.enter_context(tc.tile_pool(name="wtmp", bufs=2))
        ht_p = ctx.enter_context(tc.tile_pool(name="ht", bufs=3))
        t1_p = ctx.enter_context(tc.tile_pool(name="t1", bufs=2))
        t2_p = ctx.enter_context(tc.tile_pool(name="t2", bufs=1))
        nm_p = ctx.enter_context(tc.tile_pool(name="nm", bufs=1))
        idx_p = ctx.enter_context(tc.tile_pool(name="idxp", bufs=2))
        dq_p = ctx.enter_context(tc.tile_pool(name="dqp", bufs=4))
        m_p = ctx.enter_context(tc.tile_pool(name="mp", bufs=6))
        h1_p = ctx.enter_context(tc.tile_pool(name="h1p", bufs=3))
        oc_p = ctx.enter_context(tc.tile_pool(name="ocp", bufs=2))
        pp_big = ctx.enter_context(tc.tile_pool(name="ppbig", bufs=3, space="PSUM"))
        pp_hop = ctx.enter_context(tc.tile_pool(name="pphop", bufs=3, space="PSUM"))
        pp_tr = ctx.enter_context(tc.tile_pool(name="pptr", bufs=2, space="PSUM"))

        # ---- constants ----
        iota_i = consts.tile([P, P], I32, tag="iotai")
        nc.gpsimd.iota(iota_i[:], pattern=[[1, P]], base=0, channel_multiplier=0)
        iota_f = consts.tile([P, P], F16, tag="iotaf")
        nc.vector.tensor_copy(iota_f[:], iota_i[:])
        ident = consts.tile([P, P], F16, tag="ident")
        make_identity(nc, ident[:])
        zeroM = consts.tile([P, P], F16, tag="zerom")
        nc.vector.memset(zeroM[:], 0.0)

        # Phase-A-scoped pools: released before v_p is allocated so the
        # gather value buffers can use their SBUF.
        xt_p = tc.alloc_tile_pool(name="xtp", bufs=1)
        cast_p = tc.alloc_tile_pool(name="castp", bufs=2)

        def load_cast(dst, src_ap, n):
            c0 = 0
            while c0 < n:
                cw = min(512, n - c0)
                tmp = cast_p.tile([P, 512], F32, tag="cast")
                nc.sync.dma_start(out=tmp[:, 0:cw], in_=src_ap[:, c0:c0 + cw])
                nc.any.tensor_copy(dst[:, c0:c0 + cw], tmp[:, 0:cw])
                c0 += cw

        # x^T cast to fp16
        xT0 = xt_p.tile([P, NLOC], F16, tag="xt0")
        xT1 = xt_p.tile([P, NLOC], F16, tag="xt1")
        load_cast(xT0, xT_d[0:P, :], NLOC)
        load_cast(xT1, xT_d[P:2 * P, :], NLOC)

        # weights (cast fp16); biases fp32
        W1a, W1b, W2sb, b1c, b2c = [], [], [], [], []
        for r in range(R):
            wa = consts.tile([P, H], F16, tag=f"w1a{r}")
            wb = consts.tile([P, H], F16, tag=f"w1b{r}")
            w2 = consts.tile([P, H], F16, tag=f"w2{r}")
            load_cast(wa, Wd[f"W1_{r}"][0:P, :], H)
            load_cast(wb, Wd[f"W1_{r}"][P:2 * P, :], H)
            load_cast(w2, Wd[f"W2_{r}"][:, :], H)
            b1 = consts.tile([P, 1], F32, tag=f"b1{r}")
            b2 = consts.tile([P, 1], F32, tag=f"b2{r}")
            nc.sync.dma_start(out=b1[:], in_=Wd[f"b1_{r}"][:, None])
            nc.sync.dma_start(out=b2[:], in_=Wd[f"b2_{r}"][:, None])
            W1a.append(wa); W1b.append(wb); W2sb.append(w2)
            b1c.append(b1); b2c.append(b2)

        # W3 folded by Bernstein thetas: W3p_k = sum_j THETA[j,k] * W3_j
        w3s = []
        for jj in range(KORD):
            t = wtmp_p.tile([P, H], F32, tag=f"w3s{jj}")
            nc.sync.dma_start(out=t[:], in_=W3_d[jj * H:(jj + 1) * H, :])
            w3s.append(t)
        W3p = []
        for k in range(KORD):
            acc = wtmp_p.tile([P, H], F32, tag=f"w3acc{k}")
            nc.vector.tensor_scalar(out=acc[:], in0=w3s[0][:],
                                    scalar1=float(THETAS[0, k]), scalar2=None,
                                    op0=mybir.AluOpType.mult)
            for jj in range(1, KORD):
                t2t = wtmp_p.tile([P, H], F32, tag="w3mul")
                nc.vector.tensor_scalar(out=t2t[:], in0=w3s[jj][:],
                                        scalar1=float(THETAS[jj, k]), scalar2=None,
                                        op0=mybir.AluOpType.mult)
                nc.vector.tensor_tensor(out=acc[:], in0=acc[:], in1=t2t[:],
                                        op=mybir.AluOpType.add)
            wk = consts.tile([P, H], F16, tag=f"w3p{k}")
            nc.vector.tensor_copy(wk[:], acc[:])
            W3p.append(wk)
        b3x3 = consts.tile([P, 1], F32, tag="b3x3")
        nc.sync.dma_start(out=b3x3[:], in_=b3_d[:, None])
        nc.vector.tensor_scalar(out=b3x3[:], in0=b3x3[:], scalar1=3.0,
                                scalar2=None, op0=mybir.AluOpType.mult)

        out_acc = consts.tile([P, NLOC], F16, tag="outacc")

        def transpose_chunk(src_fm, nm_tile, k):
            """Transpose windows of chunk k from feat-major src into node-major
            nm_tile [P, nw*P]."""
            w0, nw = chunks[k]
            for j in range(nw):
                tp = pp_tr.tile([P, P], F16, space="PSUM", tag="tr")
                nc.tensor.transpose(out=tp[:],
                                    in_=src_fm[:, (w0 + j) * P:(w0 + j + 1) * P],
                                    identity=ident[:])
                nc.any.tensor_copy(nm_tile[:, j * P:(j + 1) * P], tp[:])

        def allgather_chunk(src_fm, ag_in_k, table_k, k):
            """Transpose chunk k of feat-major src, store to DRAM, AllGather."""
            w0, nw = chunks[k]
            nm_tile = nm_p.tile([P, max(nw for _, nw in chunks) * P], F16,
                                tag="nm")
            transpose_chunk(src_fm, nm_tile, k)
            nc.sync.dma_start(
                out=ag_in_k.ap().rearrange("(j p) f -> p j f", p=P),
                in_=nm_tile[:, 0:nw * P].rearrange("p (j f) -> p j f", f=H))
            if os.environ.get("KSKIP_AG"):
                return
            nc.gpsimd.collective_compute(
                "AllGather", mybir.AluOpType.bypass,
                ins=[ag_in_k.ap()], outs=[table_k.ap()],
                replica_groups=[list(range(NCORES))])

        def hop(r, T, tables, prev_fm, next_fm, idx_sb, dq_sb, wq_sb,
                after_group=None):
            """next_fm = prev_fm - A_hat @ gathered(tables).

            after_group(ci) is called after all segs of seg_groups[ci] have
            their epilogue emitted (used to launch the next AG per chunk).
            """
            kmode = os.environ.get("KMODE", "full")
            if kmode == "nohop":
                nc.any.tensor_copy(next_fm[:], prev_fm[:])
                if after_group is not None:
                    for ci in range(K):
                        after_group(ci)
                return
            # stream column offset per (window, chunk), matching host layout
            stream_off = {}
            tpos = 0
            for seg in segs:
                for k in range(K):
                    for jj in seg:
                        stream_off[(jj, k)] = tpos
                        tpos += int(T[jj, k])
            single_pkt = bool(os.environ.get("KSP"))
            icol = 0
            for ci, sg in enumerate(seg_groups):
                for si in sg:
                    seg = segs[si]
                    vbufs = {}
                    slot0 = {}
                    for k in range(K):
                        tcount = int(T[seg, k].sum())
                        if tcount == 0:
                            continue
                        vb = v_p.tile([P, maxcall * P], F16, tag="vbuf")
                        if kmode == "nogather":
                            nc.vector.memset(vb[:, 0:tcount * P], 0.0)
                        else:
                            nc._gq = getattr(nc, "_gq", -1) + 1
                            nc.gpsimd.dma_gather(
                                out_ap=vb[:, 0:tcount * P].rearrange(
                                    "p (t e) -> p t e", e=P),
                                in_ap=tables[k].ap(),
                                idxs_ap=idx_sb[:, icol:icol + tcount * 8],
                                num_idxs=tcount * P,
                                num_idxs_reg=tcount * P,
                                elem_size=H,
                                single_packet=single_pkt,
                                queue_num=nc._gq % nc.num_swdge_queues,
                            )
                        icol += tcount * 8
                        vbufs[k] = vb
                        s = 0
                        for jj in seg:
                            slot0[(jj, k)] = s
                            s += int(T[jj, k])
                    j0 = seg[0]
                    bw = len(seg)
                    ps = pp_hop.tile([P, WIN_PER_SEG * P], F32, space="PSUM",
                                     tag="hop")
                    for jj in seg:
                        tot = int(T[jj].sum())
                        reg = ps[:, (jj - j0) * P:(jj - j0 + 1) * P]
                        if tot == 0:
                            nc.tensor.matmul(out=reg, lhsT=zeroM[:],
                                             rhs=zeroM[:], start=True, stop=True)
                            continue
                        kk = 0
                        for k in range(K):
                            for t in range(int(T[jj, k])):
                                col = stream_off[(jj, k)] + t
                                m = m_p.tile([P, P], F16, tag="onehot")
                                nc.any.tensor_scalar(
                                    out=m[:], in0=iota_f[:],
                                    scalar1=dq_sb[:, col:col + 1],
                                    scalar2=wq_sb[:, col:col + 1],
                                    op0=mybir.AluOpType.is_equal,
                                    op1=mybir.AluOpType.mult)
                                nc.tensor.matmul(
                                    out=reg,
                                    lhsT=vbufs[k][:, (slot0[(jj, k)] + t) * P:
                                                  (slot0[(jj, k)] + t + 1) * P],
                                    rhs=m[:],
                                    start=(kk == 0), stop=(kk == tot - 1))
                                kk += 1
                    nc.any.tensor_tensor(
                        out=next_fm[:, j0 * P:(j0 + bw) * P],
                        in0=prev_fm[:, j0 * P:(j0 + bw) * P],
                        in1=ps[:, 0:bw * P],
                        op=mybir.AluOpType.subtract)
                if after_group is not None:
                    after_group(ci)

        # ---- relations (software-pipelined) ----
        # Phase A: all MLPs, with chunked AG of h as each chunk completes.
        hTs = []
        for r in range(R):
            hT = ht_p.tile([P, NLOC], F16, tag="ht")
            for k, (w0, nw) in enumerate(chunks):
                c0 = w0 * P
                cend = (w0 + nw) * P
                while c0 < cend:
                    cw = min(MLP_CHUNK, cend - c0)
                    ps1 = pp_big.tile([P, MLP_CHUNK], F32, space="PSUM", tag="big")
                    nc.tensor.matmul(out=ps1[:, 0:cw], lhsT=W1a[r][:],
                                     rhs=xT0[:, c0:c0 + cw], start=True, stop=False)
                    nc.tensor.matmul(out=ps1[:, 0:cw], lhsT=W1b[r][:],
                                     rhs=xT1[:, c0:c0 + cw], start=False, stop=True)
                    h1 = h1_p.tile([P, MLP_CHUNK], F16, tag="h1")
                    nc.scalar.activation(h1[:, 0:cw], ps1[:, 0:cw],
                                         mybir.ActivationFunctionType.Lrelu,
                                         bias=b1c[r][:], scale=1.0, alpha=0.01)
                    ps2 = pp_big.tile([P, MLP_CHUNK], F32, space="PSUM", tag="big")
                    nc.tensor.matmul(out=ps2[:, 0:cw], lhsT=W2sb[r][:],
                                     rhs=h1[:, 0:cw], start=True, stop=True)
                    nc.scalar.activation(hT[:, c0:c0 + cw], ps2[:, 0:cw],
                                         mybir.ActivationFunctionType.Lrelu,
                                         bias=b2c[r][:], scale=1.0, alpha=0.01)
                    c0 += cw
                allgather_chunk(hT, aghin[r][k], htab[r][k], k)
            hTs.append(hT)

        cast_p.release()
        xt_p.release()
        v_p = ctx.enter_context(
            tc.tile_pool(name="vp", bufs=int(os.environ.get("KVBUF", "4"))))

        # Phase B: hops + projection, interleaved across relations so each
        # AllGather transfer hides behind a full hop of another relation.
        streams_sb = {}

        def load_streams(r):
            idx_sb = idx_p.tile([P, max_ntiles * 8], I16, tag="idx")
            nc.sync.dma_start(out=idx_sb[:, 0:ntiles[r] * 8], in_=idx_d[r][:])
            dq_sb = dq_p.tile([P, max_ntiles], F32, tag="dq")
            wq_sb = dq_p.tile([P, max_ntiles], F32, tag="wq")
            nc.sync.dma_start(out=dq_sb[:, 0:ntiles[r]], in_=dq_d[r][:])
            nc.sync.dma_start(out=wq_sb[:, 0:ntiles[r]], in_=wq_d[r][:])
            streams_sb[r] = (idx_sb, dq_sb, wq_sb)

        T1s, T2s = {}, {}

        def hop1(r):
            load_streams(r)
            idx_sb, dq_sb, wq_sb = streams_sb[r]
            T1 = t1_p.tile([P, NLOC], F16, tag="t1")
            T1s[r] = T1

            def ag_t1(ci):
                allgather_chunk(T1, agtin[r][ci], ttab[r][ci], ci)

            hop(r, Ts[r], htab[r], hTs[r], T1, idx_sb, dq_sb, wq_sb,
                after_group=ag_t1)

        def hop2(r):
            idx_sb, dq_sb, wq_sb = streams_sb[r]
            T2 = t2_p.tile([P, NLOC], F16, tag="t2")
            T2s[r] = T2
            hop(r, Ts[r], ttab[r], T1s[r], T2, idx_sb, dq_sb, wq_sb)

        def proj(r):
            hT, T1, T2 = hTs[r], T1s[r], T2s[r]
            for c0 in range(0, NLOC, MLP_CHUNK):
                cw = min(MLP_CHUNK, NLOC - c0)
                psf = pp_big.tile([P, MLP_CHUNK], F32, space="PSUM", tag="big")
                nc.tensor.matmul(out=psf[:, 0:cw], lhsT=W3p[0][:],
                                 rhs=hT[:, c0:c0 + cw], start=True, stop=False)
                nc.tensor.matmul(out=psf[:, 0:cw], lhsT=W3p[1][:],
                                 rhs=T1[:, c0:c0 + cw], start=False, stop=False)
                nc.tensor.matmul(out=psf[:, 0:cw], lhsT=W3p[2][:],
                                 rhs=T2[:, c0:c0 + cw], start=False, stop=True)
                if r == 0:
                    nc.any.tensor_copy(out_acc[:, c0:c0 + cw], psf[:, 0:cw])
                else:
                    nc.any.tensor_tensor(out=out_acc[:, c0:c0 + cw],
                                         in0=out_acc[:, c0:c0 + cw],
                                         in1=psf[:, 0:cw],
                                         op=mybir.AluOpType.add)

        hop1(0)
        hop1(1)
        hop2(0)
        proj(0)
        hop1(2)
        hop2(1)
        proj(1)
        hop2(2)
        proj(2)

        # ---- output: leaky(out_acc + 3*b3), feat-major ----
        for c0 in range(0, NLOC, MLP_CHUNK):
            cw = min(MLP_CHUNK, NLOC - c0)
            oc = oc_p.tile([P, MLP_CHUNK], F32, tag="oc")
            nc.scalar.activation(oc[:, 0:cw], out_acc[:, c0:c0 + cw],
                                 mybir.ActivationFunctionType.Lrelu,
                                 bias=b3x3[:], scale=1.0, alpha=0.01)
            nc.sync.dma_start(out=out_d[:, c0:c0 + cw], in_=oc[:, 0:cw])

    nc.compile()
    return nc


# ----------------------------------------------------------------------------
# Entry point
# ----------------------------------------------------------------------------

_prog_cache = {}


def kernel(**inputs):
    meta, in_maps, perm = preprocess(inputs)
    key = repr((meta["N"], meta["NLOC"], meta["T"], meta["ntiles"], NCHUNK))
    if key not in _prog_cache:
        _prog_cache[key] = build_program(meta)
    nc = _prog_cache[key]
    res = run_bass_kernel_spmd(nc, in_maps, list(range(NCORES)))
    outs = [res.results[c]["out"] for c in range(NCORES)]  # [P, NLOC] each
    out_slots = np.concatenate(outs, axis=1).T             # [NPAD, H]
    n = meta["N"]
    return np.ascontiguousarray(out_slots[perm[:n]]).astype(np.float32)


# revision 16
# speedup vs baseline: 1.8400x; 1.2106x over previous
"""Trainium2 Bass kernel for nn_CombinedModel (3-relation GNN with Bernstein
polynomial message passing).

Self-contained: takes full inputs, shards nodes across 8 NeuronCores,
runs a Bass/Tile SPMD program (MLP -> 2 hops of normalized-Laplacian
aggregation -> polynomial projection), gathers the full output.

The node table used by the hop gathers is AllGathered in K chunks so the
collective pipelines against MLP / gather / matmul compute instead of
serializing in front of each hop.
"""
import math
import os
from contextlib import ExitStack

import numpy as np

import concourse.bacc as bacc
import concourse.tile as tile
from concourse import mybir
from concourse.bass_utils import run_bass_kernel_spmd
from concourse.masks import make_identity

F16, F32 = mybir.dt.float16, mybir.dt.float32
I16, I32 = mybir.dt.int16, mybir.dt.int32

NCORES = 8
P = 128
H = 128
IN_FEATS = 256
R = 3
D_ORDER = 2
KORD = D_ORDER + 1
WIN_PER_SEG = 4     # windows per gather segment == windows per PSUM bank
MLP_CHUNK = 512
NCHUNK = int(os.environ.get("KCHUNKS", "2"))   # table / AllGather chunks


def _bernstein_thetas(d):
    thetas = []
    for i in range(d + 1):
        a = np.zeros(i + 1)
        a[i] = 0.5 ** i
        b = np.array([math.comb(d - i, j) * (-0.5) ** j for j in range(d - i + 1)])
        scale = math.factorial(d + 1) / (math.factorial(i) * math.factorial(d - i))
        thetas.append((np.convolve(a, b) * scale).astype(np.float32))
    return np.stack(thetas)  # [d+1, d+1]


THETAS = _bernstein_thetas(D_ORDER)


# ----------------------------------------------------------------------------
# Host-side preprocessing
# ----------------------------------------------------------------------------

def _make_plan(n):
    nloc = ((n + NCORES * P - 1) // (NCORES * P)) * P
    npad = nloc * NCORES
    nwin = nloc // P
    segs = [list(range(s, min(s + WIN_PER_SEG, nwin)))
            for s in range(0, nwin, WIN_PER_SEG)]
    # chunks: K groups of consecutive segs, window counts as equal as possible
    nseg = len(segs)
    bounds = [round(i * nseg / NCHUNK) for i in range(NCHUNK + 1)]
    seg_groups = [list(range(bounds[i], bounds[i + 1])) for i in range(NCHUNK)]
    chunks = []  # (win_start, win_count) per chunk
    for sg in seg_groups:
        w0 = segs[sg[0]][0]
        w1 = segs[sg[-1]][-1] + 1
        chunks.append((w0, w1 - w0))
    assert all(nw * P * NCORES < 32768 for _, nw in chunks)
    return dict(N=n, NLOC=nloc, NPAD=npad, NWIN=nwin, segs=segs,
                chunks=chunks, seg_groups=seg_groups)


def _build_perm(degs_total, npad):
    """Balanced node -> slot permutation. Snake-deal nodes (sorted by total
    degree desc) across all (core, window) bins so per-window edge counts are
    near-equal across cores."""
    n = len(degs_total)
    nloc = npad // NCORES
    nwin_total = npad // P
    tot = np.zeros(npad, np.int64)
    tot[:n] = degs_total
    order = np.argsort(-tot, kind="stable")
    slot_of = np.empty(npad, np.int64)
    counts = np.zeros(nwin_total, np.int64)
    i = np.arange(npad)
    rnd, pos = np.divmod(i, nwin_total)
    w = np.where(rnd % 2 == 0, pos, nwin_total - 1 - pos)
    core = w % NCORES
    j = w // NCORES
    for idx in range(npad):
        g = order[idx]
        ww = w[idx]
        slot_of[g] = core[idx] * nloc + j[idx] * P + counts[ww]
        counts[ww] += 1
    return slot_of


def _pack_idx(idx_flat):
    """[L] int16 -> wrapped [128, L//16] layout (16-partition wrap, replicated)."""
    L = len(idx_flat)
    assert L % 16 == 0
    base = idx_flat.reshape(L // 16, 16).T  # [16, L/16]
    return np.ascontiguousarray(np.tile(base, (8, 1))).astype(np.int16)


def _build_streams(plan, s_slot, d_slot, wgt):
    """Build per-core gather/one-hot streams for one relation.

    Edges are grouped by (dst core, dst window, src chunk); each group is
    padded to whole 128-edge tiles. Gather indices address the per-chunk
    AllGathered table [NCORES * chunk_wins * P, H].

    Stream tile order: for seg in segs: for k in chunks: for j in seg.
    """
    NLOC, NWIN = plan["NLOC"], plan["NWIN"]
    chunks = plan["chunks"]
    K = len(chunks)
    core = d_slot // NLOC
    j = (d_slot % NLOC) // P
    off = (d_slot % P).astype(np.float32)

    s_core = s_slot // NLOC
    s_loc = s_slot % NLOC
    s_win = s_loc // P
    win_chunk = np.zeros(NWIN, np.int64)
    win_local = np.zeros(NWIN, np.int64)
    for k, (w0, nw) in enumerate(chunks):
        win_chunk[w0:w0 + nw] = k
        win_local[w0:w0 + nw] = np.arange(nw)
    part = win_chunk[s_win]
    crows = np.array([nw * P for _, nw in chunks])
    idx_vals = s_core * crows[part] + win_local[s_win] * P + (s_loc % P)

    key = (core * NWIN + j) * K + part
    ngroups = NCORES * NWIN * K
    cnt = np.bincount(key, minlength=ngroups).reshape(NCORES, NWIN, K)
    T = np.ceil(cnt / P).astype(np.int64).max(axis=0)  # [NWIN, K]
    order = np.argsort(key, kind="stable")
    gstart = np.zeros(ngroups + 1, np.int64)
    np.cumsum(np.bincount(key, minlength=ngroups), out=gstart[1:])

    ntiles = int(T.sum())
    tile_off = {}
    tpos = 0
    for seg in plan["segs"]:
        for k in range(K):
            for jj in seg:
                tile_off[(jj, k)] = tpos
                tpos += int(T[jj, k])
    assert tpos == ntiles

    # sort edges within each group by source index (HBM row locality)
    order = order[np.lexsort((idx_vals[order], key[order]))]
    per_core = []
    for c in range(NCORES):
        idx_c = np.zeros(ntiles * P, np.int64)
        dq_c = np.full(ntiles * P, -1.0, np.float32)
        wq_c = np.zeros(ntiles * P, np.float32)
        for jj in range(NWIN):
            for k in range(K):
                g = (c * NWIN + jj) * K + k
                e = order[gstart[g]:gstart[g + 1]]
                L = len(e)
                if L == 0:
                    continue
                s0 = tile_off[(jj, k)] * P
                idx_c[s0:s0 + L] = idx_vals[e]
                dq_c[s0:s0 + L] = off[e]
                wq_c[s0:s0 + L] = wgt[e]
        per_core.append(dict(
            idx=_pack_idx(idx_c.astype(np.int16)),
            dq=np.ascontiguousarray(dq_c.reshape(ntiles, P).T),
            wq=np.ascontiguousarray(wq_c.reshape(ntiles, P).T),
        ))
    return T, ntiles, per_core


def preprocess(inputs):
    x = np.asarray(inputs["x"], np.float32)
    n = x.shape[0]
    plan = _make_plan(n)
    NLOC, NPAD = plan["NLOC"], plan["NPAD"]

    srcs, dsts, degs = [], [], []
    for r in range(R):
        s = np.asarray(inputs[f"src{r}"]).astype(np.int64)
        d = np.asarray(inputs[f"dst{r}"]).astype(np.int64)
        srcs.append(s)
        dsts.append(d)
        degs.append(np.bincount(d, minlength=n).astype(np.float64))
    perm = _build_perm(sum(degs)[:n].astype(np.int64), NPAD)  # global -> slot

    meta = dict(N=n, NLOC=NLOC, NPAD=NPAD, NWIN=plan["NWIN"],
                segs=tuple(tuple(s) for s in plan["segs"]),
                chunks=tuple(tuple(c) for c in plan["chunks"]),
                seg_groups=tuple(tuple(g) for g in plan["seg_groups"]))
    Ts, ntiles_l, streams = [], [], []
    for r in range(R):
        dinv = 1.0 / np.sqrt(np.maximum(degs[r], 1.0))
        wgt = (dinv[srcs[r]] * dinv[dsts[r]]).astype(np.float32)
        T, ntiles, per_core = _build_streams(
            plan, perm[srcs[r]], perm[dsts[r]], wgt)
        Ts.append(tuple(tuple(int(v) for v in row) for row in T))
        ntiles_l.append(ntiles)
        streams.append(per_core)
    meta["T"] = tuple(Ts)
    meta["ntiles"] = tuple(ntiles_l)

    x_slots = np.zeros((NPAD, IN_FEATS), np.float32)
    x_slots[perm[:n]] = x
    in_maps = []
    weight_names = []
    for r in range(R):
        weight_names += [f"W1_{r}", f"b1_{r}", f"W2_{r}", f"b2_{r}"]
    weight_names += ["W3", "b3"]
    for c in range(NCORES):
        m = {"xT": np.ascontiguousarray(
            x_slots[c * NLOC:(c + 1) * NLOC].T)}
        for name in weight_names:
            m[name] = np.asarray(inputs[name], np.float32)
        for r in range(R):
            m[f"idx{r}"] = streams[r][c]["idx"]
            m[f"dq{r}"] = streams[r][c]["dq"]
            m[f"wq{r}"] = streams[r][c]["wq"]
        in_maps.append(m)
    return meta, in_maps, perm


# ----------------------------------------------------------------------------
# Device program
# ----------------------------------------------------------------------------

def build_program(meta):
    NLOC, NPAD, NWIN = meta["NLOC"], meta["NPAD"], meta["NWIN"]
    segs = [list(s) for s in meta["segs"]]
    chunks = [tuple(c) for c in meta["chunks"]]
    seg_groups = [list(g) for g in meta["seg_groups"]]
    K = len(chunks)
    Ts = [np.array(t, np.int64) for t in meta["T"]]
    ntiles = meta["ntiles"]

    # gather calls are split to at most MAXT tiles (rows/call near the
    # measured SWDGE sweet spot), balanced within a (seg, chunk) group
    MAXT = int(os.environ.get("KMAXT", "24"))
    max_ntiles = max(ntiles)

    NQ = int(os.environ.get("KQ", "4"))
    nc = bacc.Bacc("TRN2", target_bir_lowering=False, debug=False,
                   num_devices=NCORES, num_swdge_queues=NQ)

    xT_d = nc.dram_tensor("xT", [IN_FEATS, NLOC], F32, kind="ExternalInput").ap()
    Wd = {}
    for r in range(R):
        Wd[f"W1_{r}"] = nc.dram_tensor(f"W1_{r}", [IN_FEATS, H], F32, kind="ExternalInput").ap()
        Wd[f"b1_{r}"] = nc.dram_tensor(f"b1_{r}", [H], F32, kind="ExternalInput").ap()
        Wd[f"W2_{r}"] = nc.dram_tensor(f"W2_{r}", [H, H], F32, kind="ExternalInput").ap()
        Wd[f"b2_{r}"] = nc.dram_tensor(f"b2_{r}", [H], F32, kind="ExternalInput").ap()
    W3_d = nc.dram_tensor("W3", [KORD * H, H], F32, kind="ExternalInput").ap()
    b3_d = nc.dram_tensor("b3", [H], F32, kind="ExternalInput").ap()
    idx_d, dq_d, wq_d = [], [], []
    for r in range(R):
        idx_d.append(nc.dram_tensor(f"idx{r}", [P, ntiles[r] * 8], I16, kind="ExternalInput").ap())
        dq_d.append(nc.dram_tensor(f"dq{r}", [P, ntiles[r]], F32, kind="ExternalInput").ap())
        wq_d.append(nc.dram_tensor(f"wq{r}", [P, ntiles[r]], F32, kind="ExternalInput").ap())
    out_d = nc.dram_tensor("out", [P, NLOC], F32, kind="ExternalOutput").ap()

    # internal DRAM: per-chunk AG inputs + shared tables, for h and T1
    aghin = [[nc.dram_tensor(f"aghin{r}_{k}", [nw * P, H], F16)
              for k, (w0, nw) in enumerate(chunks)] for r in range(R)]
    htab = [[nc.dram_tensor(f"htab{r}_{k}", [NCORES * nw * P, H], F16,
                            addr_space="Shared")
             for k, (w0, nw) in enumerate(chunks)] for r in range(R)]
    agtin = [[nc.dram_tensor(f"agtin{r}_{k}", [nw * P, H], F16)
              for k, (w0, nw) in enumerate(chunks)] for r in range(R)]
    ttab = [[nc.dram_tensor(f"ttab{r}_{k}", [NCORES * nw * P, H], F16,
                            addr_space="Shared")
             for k, (w0, nw) in enumerate(chunks)] for r in range(R)]

    with tile.TileContext(nc) as tc, ExitStack() as ctx:
        consts = ctx.enter_context(tc.tile_pool(name="consts", bufs=1))
        wtmp_p = ctx.enter_context(tc.tile_pool(name="wtmp", bufs=2))
        ht_p = ctx.enter_context(tc.tile_pool(name="ht", bufs=3))
        t1_p = ctx.enter_context(tc.tile_pool(name="t1", bufs=2))
        t2_p = ctx.enter_context(tc.tile_pool(name="t2", bufs=1))
        nm_p = ctx.enter_context(tc.tile_pool(name="nm", bufs=1))
        idx_p = ctx.enter_context(tc.tile_pool(name="idxp", bufs=2))
        dq_p = ctx.enter_context(tc.tile_pool(name="dqp", bufs=4))
        m_p = ctx.enter_context(tc.tile_pool(name="mp", bufs=6))
        h1_p = ctx.enter_context(tc.tile_pool(name="h1p", bufs=2))
        oc_p = ctx.enter_context(tc.tile_pool(name="ocp", bufs=1))
        pp_big = ctx.enter_context(tc.tile_pool(name="ppbig", bufs=3, space="PSUM"))
        pp_hop = ctx.enter_context(tc.tile_pool(name="pphop", bufs=3, space="PSUM"))
        pp_tr = ctx.enter_context(tc.tile_pool(name="pptr", bufs=2, space="PSUM"))

        # ---- constants ----
        iota_i = consts.tile([P, P], I32, tag="iotai")
        nc.gpsimd.iota(iota_i[:], pattern=[[1, P]], base=0, channel_multiplier=0)
        iota_f = consts.tile([P, P], F16, tag="iotaf")
        nc.vector.tensor_copy(iota_f[:], iota_i[:])
        ident = consts.tile([P, P], F16, tag="ident")
        make_identity(nc, ident[:])
        zeroM = consts.tile([P, P], F16, tag="zerom")
        nc.vector.memset(zeroM[:], 0.0)

        # Phase-A-scoped pools: released before v_p is allocated so the
        # gather value buffers can use their SBUF.
        xt_p = tc.alloc_tile_pool(name="xtp", bufs=1)
        cast_p = tc.alloc_tile_pool(name="castp", bufs=2)

        def load_cast(dst, src_ap, n):
            c0 = 0
            while c0 < n:
                cw = min(512, n - c0)
                tmp = cast_p.tile([P, 512], F32, tag="cast")
                nc.sync.dma_start(out=tmp[:, 0:cw], in_=src_ap[:, c0:c0 + cw])
                nc.any.tensor_copy(dst[:, c0:c0 + cw], tmp[:, 0:cw])
                c0 += cw

        # x^T cast to fp16
        xT0 = xt_p.tile([P, NLOC], F16, tag="xt0")
        xT1 = xt_p.tile([P, NLOC], F16, tag="xt1")
        load_cast(xT0, xT_d[0:P, :], NLOC)
        load_cast(xT1, xT_d[P:2 * P, :], NLOC)

        # weights (cast fp16); biases fp32
        W1a, W1b, W2sb, b1c, b2c = [], [], [], [], []
        for r in range(R):
            wa = consts.tile([P, H], F16, tag=f"w1a{r}")
            wb = consts.tile([P, H], F16, tag=f"w1b{r}")
            w2 = consts.tile([P, H], F16, tag=f"w2{r}")
            load_cast(wa, Wd[f"W1_{r}"][0:P, :], H)
            load_cast(wb, Wd[f"W1_{r}"][P:2 * P, :], H)
            load_cast(w2, Wd[f"W2_{r}"][:, :], H)
            b1 = consts.tile([P, 1], F32, tag=f"b1{r}")
            b2 = consts.tile([P, 1], F32, tag=f"b2{r}")
            nc.sync.dma_start(out=b1[:], in_=Wd[f"b1_{r}"][:, None])
            nc.sync.dma_start(out=b2[:], in_=Wd[f"b2_{r}"][:, None])
            W1a.append(wa); W1b.append(wb); W2sb.append(w2)
            b1c.append(b1); b2c.append(b2)

        # W3 folded by Bernstein thetas: W3p_k = sum_j THETA[j,k] * W3_j
        w3s = []
        for jj in range(KORD):
            t = wtmp_p.tile([P, H], F32, tag=f"w3s{jj}")
            nc.sync.dma_start(out=t[:], in_=W3_d[jj * H:(jj + 1) * H, :])
            w3s.append(t)
        W3p = []
        for k in range(KORD):
            acc = wtmp_p.tile([P, H], F32, tag=f"w3acc{k}")
            nc.vector.tensor_scalar(out=acc[:], in0=w3s[0][:],
                                    scalar1=float(THETAS[0, k]), scalar2=None,
                                    op0=mybir.AluOpType.mult)
            for jj in range(1, KORD):
                t2t = wtmp_p.tile([P, H], F32, tag="w3mul")
                nc.vector.tensor_scalar(out=t2t[:], in0=w3s[jj][:],
                                        scalar1=float(THETAS[jj, k]), scalar2=None,
                                        op0=mybir.AluOpType.mult)
                nc.vector.tensor_tensor(out=acc[:], in0=acc[:], in1=t2t[:],
                                        op=mybir.AluOpType.add)
            wk = consts.tile([P, H], F16, tag=f"w3p{k}")
            nc.vector.tensor_copy(wk[:], acc[:])
            W3p.append(wk)
        b3x3 = consts.tile([P, 1], F32, tag="b3x3")
        nc.sync.dma_start(out=b3x3[:], in_=b3_d[:, None])
        nc.vector.tensor_scalar(out=b3x3[:], in0=b3x3[:], scalar1=3.0,
                                scalar2=None, op0=mybir.AluOpType.mult)

        out_acc = consts.tile([P, NLOC], F16, tag="outacc")

        def transpose_chunk(src_fm, nm_tile, k):
            """Transpose windows of chunk k from feat-major src into node-major
            nm_tile [P, nw*P]."""
            w0, nw = chunks[k]
            for j in range(nw):
                tp = pp_tr.tile([P, P], F16, space="PSUM", tag="tr")
                nc.tensor.transpose(out=tp[:],
                                    in_=src_fm[:, (w0 + j) * P:(w0 + j + 1) * P],
                                    identity=ident[:])
                nc.any.tensor_copy(nm_tile[:, j * P:(j + 1) * P], tp[:])

        def allgather_chunk(src_fm, ag_in_k, table_k, k):
            """Transpose chunk k of feat-major src, store to DRAM, AllGather."""
            w0, nw = chunks[k]
            nm_tile = nm_p.tile([P, max(nw for _, nw in chunks) * P], F16,
                                tag="nm")
            transpose_chunk(src_fm, nm_tile, k)
            nc.sync.dma_start(
                out=ag_in_k.ap().rearrange("(j p) f -> p j f", p=P),
                in_=nm_tile[:, 0:nw * P].rearrange("p (j f) -> p j f", f=H))
            if os.environ.get("KSKIP_AG"):
                return
            nc.gpsimd.collective_compute(
                "AllGather", mybir.AluOpType.bypass,
                ins=[ag_in_k.ap()], outs=[table_k.ap()],
                replica_groups=[list(range(NCORES))])

        def hop(r, T, tables, prev_fm, next_fm, idx_sb, dq_sb, wq_sb,
                after_group=None):
            """next_fm = prev_fm - A_hat @ gathered(tables).

            after_group(ci) is called after all segs of seg_groups[ci] have
            their epilogue emitted (used to launch the next AG per chunk).
            """
            kmode = os.environ.get("KMODE", "full")
            if kmode == "nohop":
                nc.any.tensor_copy(next_fm[:], prev_fm[:])
                if after_group is not None:
                    for ci in range(K):
                        after_group(ci)
                return
            # stream column offset per (window, chunk), matching host layout
            stream_off = {}
            tpos = 0
            for seg in segs:
                for k in range(K):
                    for jj in seg:
                        stream_off[(jj, k)] = tpos
                        tpos += int(T[jj, k])
            icol = 0
            for ci, sg in enumerate(seg_groups):
                for si in sg:
                    seg = segs[si]
                    # split each (seg, chunk) group into balanced gather
                    # calls of <= MAXT tiles, round-robin across queues
                    slot_map = {}   # k -> per-slot (vb_tile, local_slot)
                    for k in range(K):
                        tcount = int(T[seg, k].sum())
                        if tcount == 0:
                            continue
                        ncalls = (tcount + MAXT - 1) // MAXT
                        base, rem = divmod(tcount, ncalls)
                        sizes = [base + (1 if i < rem else 0)
                                 for i in range(ncalls)]
                        slots = []
                        for sz in sizes:
                            vb = v_p.tile([P, MAXT * P], F16, tag="vbuf")
                            if kmode == "nogather":
                                nc.vector.memset(vb[:, 0:sz * P], 0.0)
                            else:
                                nc._gq = getattr(nc, "_gq", -1) + 1
                                nc.gpsimd.dma_gather(
                                    out_ap=vb[:, 0:sz * P].rearrange(
                                        "p (t e) -> p t e", e=P),
                                    in_ap=tables[k].ap(),
                                    idxs_ap=idx_sb[:, icol:icol + sz * 8],
                                    num_idxs=sz * P,
                                    num_idxs_reg=sz * P,
                                    elem_size=H,
                                    single_packet=False,
                                    queue_num=nc._gq % nc.num_swdge_queues,
                                )
                            icol += sz * 8
                            for loc in range(sz):
                                slots.append((vb, loc))
                        slot_map[k] = slots
                    j0 = seg[0]
                    bw = len(seg)
                    ps = pp_hop.tile([P, WIN_PER_SEG * P], F32, space="PSUM",
                                     tag="hop")
                    for jj in seg:
                        tot = int(T[jj].sum())
                        reg = ps[:, (jj - j0) * P:(jj - j0 + 1) * P]
                        if tot == 0:
                            nc.tensor.matmul(out=reg, lhsT=zeroM[:],
                                             rhs=zeroM[:], start=True, stop=True)
                            continue
                        kk = 0
                        for k in range(K):
                            sbase = sum(int(T[j2, k]) for j2 in seg if j2 < jj)
                            for t in range(int(T[jj, k])):
                                col = stream_off[(jj, k)] + t
                                vb, loc = slot_map[k][sbase + t]
                                m = m_p.tile([P, P], F16, tag="onehot")
                                nc.any.tensor_scalar(
                                    out=m[:], in0=iota_f[:],
                                    scalar1=dq_sb[:, col:col + 1],
                                    scalar2=wq_sb[:, col:col + 1],
                                    op0=mybir.AluOpType.is_equal,
                                    op1=mybir.AluOpType.mult)
                                nc.tensor.matmul(
                                    out=reg,
                                    lhsT=vb[:, loc * P:(loc + 1) * P],
                                    rhs=m[:],
                                    start=(kk == 0), stop=(kk == tot - 1))
                                kk += 1
                    nc.any.tensor_tensor(
                        out=next_fm[:, j0 * P:(j0 + bw) * P],
                        in0=prev_fm[:, j0 * P:(j0 + bw) * P],
                        in1=ps[:, 0:bw * P],
                        op=mybir.AluOpType.subtract)
                if after_group is not None:
                    after_group(ci)

        # ---- relations (software-pipelined) ----
        # Phase A: all MLPs, with chunked AG of h as each chunk completes.
        hTs = []
        for r in range(R):
            hT = ht_p.tile([P, NLOC], F16, tag="ht")
            for k, (w0, nw) in enumerate(chunks):
                c0 = w0 * P
                cend = (w0 + nw) * P
                while c0 < cend:
                    cw = min(MLP_CHUNK, cend - c0)
                    ps1 = pp_big.tile([P, MLP_CHUNK], F32, space="PSUM", tag="big")
                    nc.tensor.matmul(out=ps1[:, 0:cw], lhsT=W1a[r][:],
                                     rhs=xT0[:, c0:c0 + cw], start=True, stop=False)
                    nc.tensor.matmul(out=ps1[:, 0:cw], lhsT=W1b[r][:],
                                     rhs=xT1[:, c0:c0 + cw], start=False, stop=True)
                    h1 = h1_p.tile([P, MLP_CHUNK], F16, tag="h1")
                    nc.scalar.activation(h1[:, 0:cw], ps1[:, 0:cw],
                                         mybir.ActivationFunctionType.Lrelu,
                                         bias=b1c[r][:], scale=1.0, alpha=0.01)
                    ps2 = pp_big.tile([P, MLP_CHUNK], F32, space="PSUM", tag="big")
                    nc.tensor.matmul(out=ps2[:, 0:cw], lhsT=W2sb[r][:],
                                     rhs=h1[:, 0:cw], start=True, stop=True)
                    nc.scalar.activation(hT[:, c0:c0 + cw], ps2[:, 0:cw],
                                         mybir.ActivationFunctionType.Lrelu,
                                         bias=b2c[r][:], scale=1.0, alpha=0.01)
                    c0 += cw
                allgather_chunk(hT, aghin[r][k], htab[r][k], k)
            hTs.append(hT)

        cast_p.release()
        xt_p.release()
        v_p = ctx.enter_context(
            tc.tile_pool(name="vp", bufs=int(os.environ.get("KVBUF", "8"))))

        # Phase B: hops + projection, interleaved across relations so each
        # AllGather transfer hides behind a full hop of another relation.
        streams_sb = {}

        def load_streams(r):
            idx_sb = idx_p.tile([P, max_ntiles * 8], I16, tag="idx")
            nc.sync.dma_start(out=idx_sb[:, 0:ntiles[r] * 8], in_=idx_d[r][:])
            dq_sb = dq_p.tile([P, max_ntiles], F32, tag="dq")
            wq_sb = dq_p.tile([P, max_ntiles], F32, tag="wq")
            nc.sync.dma_start(out=dq_sb[:, 0:ntiles[r]], in_=dq_d[r][:])
            nc.sync.dma_start(out=wq_sb[:, 0:ntiles[r]], in_=wq_d[r][:])
            streams_sb[r] = (idx_sb, dq_sb, wq_sb)

        T1s, T2s = {}, {}

        def hop1(r):
            load_streams(r)
            idx_sb, dq_sb, wq_sb = streams_sb[r]
            T1 = t1_p.tile([P, NLOC], F16, tag="t1")
            T1s[r] = T1

            def ag_t1(ci):
                allgather_chunk(T1, agtin[r][ci], ttab[r][ci], ci)

            hop(r, Ts[r], htab[r], hTs[r], T1, idx_sb, dq_sb, wq_sb,
                after_group=ag_t1)

        def hop2(r):
            idx_sb, dq_sb, wq_sb = streams_sb[r]
            T2 = t2_p.tile([P, NLOC], F16, tag="t2")
            T2s[r] = T2
            hop(r, Ts[r], ttab[r], T1s[r], T2, idx_sb, dq_sb, wq_sb)

        def proj(r):
            hT, T1, T2 = hTs[r], T1s[r], T2s[r]
            for c0 in range(0, NLOC, MLP_CHUNK):
                cw = min(MLP_CHUNK, NLOC - c0)
                psf = pp_big.tile([P, MLP_CHUNK], F32, space="PSUM", tag="big")
                nc.tensor.matmul(out=psf[:, 0:cw], lhsT=W3p[0][:],
                                 rhs=hT[:, c0:c0 + cw], start=True, stop=False)
                nc.tensor.matmul(out=psf[:, 0:cw], lhsT=W3p[1][:],
                                 rhs=T1[:, c0:c0 + cw], start=False, stop=False)
                nc.tensor.matmul(out=psf[:, 0:cw], lhsT=W3p[2][:],
                                 rhs=T2[:, c0:c0 + cw], start=False, stop=True)
                if r == 0:
                    nc.any.tensor_copy(out_acc[:, c0:c0 + cw], psf[:, 0:cw])
                else:
                    nc.any.tensor_tensor(out=out_acc[:, c0:c0 + cw],
                                         in0=out_acc[:, c0:c0 + cw],
                                         in1=psf[:, 0:cw],
                                         op=mybir.AluOpType.add)

        hop1(0)
        hop1(1)
        hop2(0)
        proj(0)
        hop1(2)
        hop2(1)
        proj(1)
        hop2(2)
        proj(2)

        # ---- output: leaky(out_acc + 3*b3), feat-major ----
        for c0 in range(0, NLOC, MLP_CHUNK):
            cw = min(MLP_CHUNK, NLOC - c0)
            oc = oc_p.tile([P, MLP_CHUNK], F32, tag="oc")
            nc.scalar.activation(oc[:, 0:cw], out_acc[:, c0:c0 + cw],
                                 mybir.ActivationFunctionType.Lrelu,
                                 bias=b3x3[:], scale=1.0, alpha=0.01)
            nc.sync.dma_start(out=out_d[:, c0:c0 + cw], in_=oc[:, 0:cw])

    nc.compile()
    return nc


# ----------------------------------------------------------------------------
# Entry point
# ----------------------------------------------------------------------------

_prog_cache = {}


def kernel(**inputs):
    meta, in_maps, perm = preprocess(inputs)
    key = repr((meta["N"], meta["NLOC"], meta["T"], meta["ntiles"], NCHUNK))
    if key not in _prog_cache:
        _prog_cache[key] = build_program(meta)
    nc = _prog_cache[key]
    res = run_bass_kernel_spmd(nc, in_maps, list(range(NCORES)))
    outs = [res.results[c]["out"] for c in range(NCORES)]  # [P, NLOC] each
    out_slots = np.concatenate(outs, axis=1).T             # [NPAD, H]
    n = meta["N"]
    return np.ascontiguousarray(out_slots[perm[:n]]).astype(np.float32)
